# revision 29
# baseline (speedup 1.0000x reference)
"""RBF-kernel SVM prediction on 8 Trainium2 NeuronCores.

predictions = exp(-g*||x_i - t_j||^2) @ (alphas*y) + b,  g = 0.5

Strategy (per sharding hint): shard X rows 8-way, replicate train side.
Math is factorized as
    pred_i = exp(-g*||x_i||^2) * sum_j y_j * exp(x_i . t_j + c_j) + b
    c_j    = -g*||t_j||^2 + ln(alpha_j)
so the train-side affine terms ride per-partition biases and the
query-side factor is a per-row epilogue scale. Train points are host-sorted
by label so the +/- y_j signs become whole-tile add/sub.

Per-core engine balance (the loop is exp-throughput bound):
 - PE runs the G = Xt.X^T GEMM in fp8 DoubleRow (2x rate), plus weighted
   ones-matvecs that accumulate ~half the exp'd tiles straight into a
   persistent PSUM row (one accumulation group across the whole loop).
 - ACT does true exp for ~2/3 of tiles (PSUM source, per-partition bias).
 - DVE runs a custom fused op for the rest, computing bf16(e^x) bit
   patterns directly (Schraudolph: bits16 = clamp(A*(G+c)+B, 0, 32000)
   as uint16; the fp32 clamp precedes conversion so no NaN/Inf patterns
   can appear), plus bf16 tensor-tensor accumulates. Exps of DVE-acc'd
   tiles land pairwise in [128, 2048] tiles so one TT covers two j-tiles.
GPSIMD is left idle: it shares an SBUF port with the DVE and measured
as dragging all DVE 2x ops down to 1x when used for accumulates.
"""

import os
import sys

import numpy as np

for _p in ("/opt/trn_rl_repo", "/root/.axon_site/_ro/trn_rl_repo"):
    if os.path.isdir(_p) and _p not in sys.path:
        sys.path.append(_p)

import ml_dtypes

import concourse.bass as bass
import concourse.tile as tile
from concourse import bacc, mybir
from concourse.bass_utils import run_bass_kernel_spmd

GAMMA = 0.5
N, M, D = 8192, 8192, 256
NCORES = 8
IC = N // NCORES          # query rows per core (1024)
JT = M // 128             # j-tiles (64)
F32 = mybir.dt.float32
BF16 = mybir.dt.bfloat16
FP8 = mybir.dt.float8e4
U16 = mybir.dt.uint16
FP32_MIN_NORMAL = 1.1754944e-38

# Schraudolph constants for bf16: bits16(e^x) ~ A*x + B, clamped to [0, CLAMP_HI].
SCH_A = 184.6650784   # 128 / ln(2)
SCH_B = 16250.496     # 128 * (127 - 0.0430)
SCH_CLAMP = 32000.0   # < 0x7F80 (inf); e^x here never exceeds e^10 anyway

N_MAGIC = 20          # tiles exp'd on the DVE
N_DMA_ACC = 0         # tiles accumulated by CCE DMAs (SWDGE too slow; off)
N_WARM = 40           # PE warmup matmuls (HAM clock-gate release)
MV_TAIL = 56          # tiles >= this always accumulate via PE matvec
ACC_LAG = 2           # emit tile t's accumulate after tile t+ACC_LAG's GEMM

# Set by test harness to collect a profile; harness grading leaves it off.
TRACE = False
DEBUG_S = True            # emit the pre-mask s row for test-side validation
LAST_RESULTS = None

_EXP_OP = None


def _register_exp_op():
    """Register the custom DVE op computing bf16 exp bit patterns.

    body = min(max(Src0*C0 + C1, 0), C2), written to a uint16 tile whose
    bits, reinterpreted as bf16, approximate e^(Src0 + c) (C1 carries the
    per-partition c bias pre-scaled by A).
    """
    global _EXP_OP
    if _EXP_OP is not None:
        return _EXP_OP
    from concourse import dve_ops as dvo
    from concourse.dve_ops import DveOp
    from concourse.dve_spec import Spec, Src0, C0, C1, C2, Zero, maxx, minn, lower
    from concourse.dve_uop import DveOpSpec

    name = "EXP_BITS_U16_ANT"
    for op in dvo.OPS:
        if op.name == name:
            _EXP_OP = op
            return op
    spec = Spec(
        body=minn(maxx(Src0 * C0 + C1, Zero), C2),
        reference=lambda in0, in1, s0, s1, imm2: np.minimum(
            np.maximum(in0.astype(np.float32) * s0 + s1, 0.0), imm2
        ),
    )
    opcode = dvo._CUSTOM_DVE_ROW_BASE + len(dvo.OPS)
    shas = {}
    for ver in ("v3", "v4"):
        tmp = DveOpSpec(name=name, opcode=opcode, uops=lower(spec, ver=ver),
                        rd1_en=False)
        shas[ver] = tmp.sha(ver)
    op = DveOp(name, spec, subdim=False, uops_sha=shas)
    dvo.OPS.append(op)
    dvo.CUSTOM_DVE_SPECS[name] = spec
    dvo._SUB_OPCODE_FOR_NAME[name] = opcode
    _EXP_OP = op
    return op


def _plan_tiles(n_pos: int):
    """Per-tile engine assignment.

    Returns (magic, acc_mode) where acc_mode[t] is one of 'mv' (PE
    matvec), 'dve' (DVE TT into accs), 'stt' (mixed-sign tile).
    """
    tb = n_pos // 128 if n_pos % 128 else -1
    # Spread DVE-exp tiles over [4, JT): the first tiles stay on ACT so the
    # pipeline fill isn't gated on the DVE finishing its startup memsets.
    magic = set(4 + int(round(i * (JT - 4) / N_MAGIC)) for i in range(N_MAGIC))
    k = 4
    while len(magic) < N_MAGIC:          # dedupe fallback
        if k not in magic:
            magic.add(k)
        k += 1

    acc_mode = {}
    n_dma = 0
    for t in range(JT):
        if t == tb:
            acc_mode[t] = "stt"
        elif t % 2 == 0 or t >= MV_TAIL:
            acc_mode[t] = "mv"
        elif n_dma < N_DMA_ACC:
            acc_mode[t] = "dma"
            n_dma += 1
        else:
            acc_mode[t] = "dve"
    return magic, acc_mode


def _build_program(n_pos: int, b_is_zero: bool):
    exp_op = _register_exp_op()
    magic, acc_mode = _plan_tiles(n_pos)
    nc = bacc.Bacc()

    # fp8 DoubleRow operands: [ki, ksub, col] with d = ksub*128 + ki.
    xt_t = nc.dram_tensor("xt_t", [128, 2, M], FP8, kind="ExternalInput")
    x_t = nc.dram_tensor("x_t", [128, 2, IC], FP8, kind="ExternalInput")
    cj = nc.dram_tensor("cj", [128, JT], F32, kind="ExternalInput")
    bcol = nc.dram_tensor("bcol", [128, JT], F32, kind="ExternalInput")
    sgn = nc.dram_tensor("sgn", [128, 1], BF16, kind="ExternalInput")
    nxsq = nc.dram_tensor("nxsq", [1, IC], F32, kind="ExternalInput")
    bb = nc.dram_tensor("bb", [1, 1], F32, kind="ExternalInput")
    out = nc.dram_tensor("out", [1, IC], F32, kind="ExternalOutput")
    s_out = nc.dram_tensor("s_out", [1, IC], F32, kind="ExternalOutput")

    NCHUNK = 16           # xt column chunks so matmuls wait on small DMAs
    CW = M // NCHUNK      # 512 j-columns per chunk

    tb = n_pos // 128 if n_pos % 128 else -1   # mixed-sign boundary tile

    def tile_sign(t):
        return +1 if (t + 1) * 128 <= n_pos else -1

    with tile.TileContext(nc) as tc:
        with (
            tc.tile_pool(name="singles", bufs=1) as singles,
            tc.tile_pool(name="epool", bufs=8) as epool,
            tc.tile_pool(name="gpsum", bufs=3, space="PSUM") as gpsum,
            tc.tile_pool(name="spsum", bufs=1, space="PSUM") as spsum,
        ):
            # Resident inputs, all on the sync HWDGE queue (the scalar queue
            # executes on the ACT engine and steals exp throughput), small
            # first-needed operands ahead of the bulk xt chunks.
            x_sb = singles.tile([128, 2, IC], FP8, tag="x")
            nc.sync.dma_start(out=x_sb, in_=x_t[:, :, :])
            cj_sb = singles.tile([128, JT], F32, tag="cj")
            nc.sync.dma_start(out=cj_sb, in_=cj[:, :])
            bcol_sb = singles.tile([128, JT], F32, tag="bcol")
            nc.sync.dma_start(out=bcol_sb, in_=bcol[:, :])
            sgn_sb = singles.tile([128, 1], BF16, tag="sgn")
            nc.sync.dma_start(out=sgn_sb, in_=sgn[:, :])
            nxsq_sb = singles.tile([1, IC], F32, tag="nxsq")
            nc.sync.dma_start(out=nxsq_sb, in_=nxsq[:, :])
            b_sb = singles.tile([1, 1], F32, tag="b")
            nc.sync.dma_start(out=b_sb, in_=bb[:, :])
            xt_sb = [None] * NCHUNK
            for ck in range(NCHUNK):
                t = singles.tile([128, 2, CW], FP8, tag=f"xt_{ck}")
                nc.sync.dma_start(
                    out=t, in_=xt_t[:, :, ck * CW:(ck + 1) * CW],
                )
                xt_sb[ck] = t

            # Warm the PE while input DMAs are in flight (HAM clock gate).
            # warm_w memset is first so warmups aren't gated on other memsets.
            warm_w = singles.tile([128, 128], BF16, tag="warm_w")
            nc.vector.memset(warm_w, 0.0)
            warm_ps = spsum.tile([1, 128], F32, tag="s")
            for _ in range(N_WARM):
                nc.tensor.matmul(
                    out=warm_ps, lhsT=warm_w[:, 0:1], rhs=warm_w[:, :],
                    start=True, stop=True,
                )

            ones_sb = singles.tile([128, 1], BF16, tag="ones")
            nc.vector.memset(ones_sb, 1.0)
            mones_sb = singles.tile([128, 1], BF16, tag="mones")
            nc.vector.memset(mones_sb, -1.0)
            accs = singles.tile([128, IC], BF16, tag="accs")
            nc.vector.memset(accs, 0.0)
            acc_dp = singles.tile([128, IC], BF16, tag="acc_dp")
            nc.vector.memset(acc_dp, 0.0)
            acc_dn = singles.tile([128, IC], BF16, tag="acc_dn")
            nc.vector.memset(acc_dn, 0.0)

            # Query-side factor, computed early so ACT's table load happens
            # during the DMA window.
            e_row = singles.tile([1, IC], F32, tag="e_row")
            nc.scalar.activation(
                out=e_row, in_=nxsq_sb, func=mybir.ActivationFunctionType.Exp
            )
            # Emulate fp32 FTZ on the factor: the reference's direct
            # exp(-g*d) underflows to 0; keep the factored path bit-identical.
            m_row = singles.tile([1, IC], F32, tag="m_row")
            nc.vector.tensor_scalar(
                out=m_row, in0=e_row, scalar1=FP32_MIN_NORMAL, scalar2=None,
                op0=mybir.AluOpType.is_ge,
            )
            nc.vector.tensor_mul(e_row, e_row, m_row)

            # s_ps accumulates PE-matvec'd tiles across the whole j-loop and
            # receives the folds of acc2/accs at the end. Shares the "s"
            # PSUM slot with warm_ps; the first matvec's start=True clears it.
            s_ps = spsum.tile([1, IC], F32, tag="s")

            e_views = {}          # t -> bf16-view AP of tile t's exp output
            first_mv = [True]

            def emit_gemm(t):
                ck, col = t // 4, (t % 4) * 128
                g_ps = gpsum.tile([128, IC], F32, tag="g", name=f"g{t}")
                for ic in range(2):
                    sl = slice(ic * 512, (ic + 1) * 512)
                    nc.tensor.matmul(
                        out=g_ps[:, sl],
                        lhsT=xt_sb[ck][:, :, col:col + 128],
                        rhs=x_sb[:, :, sl],
                        start=True, stop=True,
                        perf_mode=mybir.MatmulPerfMode.DoubleRow,
                    )
                return g_ps

            def emit_exp(t, g_ps):
                dest = epool.tile([128, IC], BF16, tag="e", name=f"e{t}")
                if t in magic:
                    nc.vector._custom_dve(
                        exp_op, out=dest.bitcast(U16), in0=g_ps,
                        s0=SCH_A, s1=bcol_sb[:, t:t + 1], imm2=SCH_CLAMP,
                    )
                else:
                    nc.scalar.activation(
                        out=dest, in_=g_ps,
                        func=mybir.ActivationFunctionType.Exp,
                        bias=cj_sb[:, t:t + 1], scale=1.0,
                    )
                e_views[t] = dest

            def emit_acc(t):
                mode = acc_mode[t]
                e_t = e_views.pop(t)
                if mode == "dve":
                    if tile_sign(t) > 0:
                        nc.vector.tensor_add(accs, accs, e_t)
                    else:
                        nc.vector.tensor_sub(accs, accs, e_t)
                elif mode == "dma":
                    # CCE read-modify-write accumulate in the DMA engine;
                    # gpsimd only generates descriptors.
                    dst = acc_dp if tile_sign(t) > 0 else acc_dn
                    nc.gpsimd.dma_start(
                        out=dst, in_=e_t, accum_op=mybir.AluOpType.add,
                    )
                elif mode == "stt":
                    nc.vector.scalar_tensor_tensor(
                        out=accs, in0=e_t, scalar=sgn_sb[:, 0:1], in1=accs,
                        op0=mybir.AluOpType.mult, op1=mybir.AluOpType.add,
                    )
                else:
                    w = ones_sb if tile_sign(t) > 0 else mones_sb
                    for ic in range(2):
                        sl = slice(ic * 512, (ic + 1) * 512)
                        nc.tensor.matmul(
                            out=s_ps[:, sl], lhsT=w, rhs=e_t[:, sl],
                            start=first_mv[0], stop=False,
                            skip_group_check=True,
                        )
                    first_mv[0] = False

            for t in range(JT):
                g_ps = emit_gemm(t)
                emit_exp(t, g_ps)
                if t >= ACC_LAG:
                    emit_acc(t - ACC_LAG)
            for t in range(JT - ACC_LAG, JT):
                emit_acc(t)

            # Fold the DVE/DMA accumulators in: s[0,i] += sum_p acc[p,i];
            # the last matmul stops the PSUM accumulation group.
            folds = [(accs, ones_sb), (acc_dp, ones_sb), (acc_dn, mones_sb)]
            for fi, (f, w) in enumerate(folds):
                for ic in range(2):
                    sl = slice(ic * 512, (ic + 1) * 512)
                    last = fi == len(folds) - 1 and ic == 1
                    nc.tensor.matmul(
                        out=s_ps[:, sl], lhsT=w, rhs=f[:, sl],
                        start=first_mv[0], stop=last,
                        skip_group_check=True,
                    )
                    first_mv[0] = False
            if DEBUG_S:
                s_sb = singles.tile([1, IC], F32, tag="s_sb")
                nc.vector.tensor_copy(s_sb, s_ps)
                nc.sync.dma_start(out=s_out[:, :], in_=s_sb)  # pre-mask s
            p_row = singles.tile([1, IC], F32, tag="p_row")
            nc.vector.tensor_mul(p_row, s_ps, e_row)
            if not b_is_zero:
                nc.vector.tensor_scalar(
                    out=p_row, in0=p_row, scalar1=b_sb[0:1, 0:1], scalar2=None,
                    op0=mybir.AluOpType.add,
                )
            nc.sync.dma_start(out=out[:, :], in_=p_row)

    nc.finalize()
    return nc


def kernel(X, X_train, alphas, y_train, b):
    X = np.ascontiguousarray(np.asarray(X, dtype=np.float32))
    X_train = np.ascontiguousarray(np.asarray(X_train, dtype=np.float32))
    alphas = np.asarray(alphas, dtype=np.float32).reshape(M)
    y_train = np.asarray(y_train, dtype=np.float32).reshape(M)
    b_arr = np.asarray(b, dtype=np.float32).reshape(1, 1)

    # Sort train points by label (+1 first), then by c within each label so
    # c values on a partition row are close (enables shared-bias tricks).
    c_all = (-GAMMA * (X_train * X_train).sum(1)
             + np.log(np.maximum(alphas, np.float32(1e-38)))).astype(np.float32)
    perm = np.lexsort((c_all, -y_train))
    n_pos = int((y_train > 0).sum())
    Xt_p = X_train[perm]
    c = c_all[perm]

    cj = np.ascontiguousarray(c.reshape(JT, 128).T)          # [128, JT]
    bcol = (SCH_A * cj + np.float32(SCH_B)).astype(np.float32)
    r = n_pos % 128
    sgn_vec = np.where(np.arange(128) < r, 1.0, -1.0).astype(
        ml_dtypes.bfloat16).reshape(128, 1)

    # fp8 DoubleRow layouts: [ki, ksub, col], d = ksub*128 + ki.
    f8 = ml_dtypes.float8_e4m3fn
    xt_dr = np.ascontiguousarray(
        Xt_p.T.reshape(2, 128, M).transpose(1, 0, 2).astype(f8))
    nxsq_full = (-GAMMA * (X * X).sum(1)).astype(np.float32)

    in_maps = []
    for k in range(NCORES):
        sl = slice(k * IC, (k + 1) * IC)
        x_dr = np.ascontiguousarray(
            X[sl].T.reshape(2, 128, IC).transpose(1, 0, 2).astype(f8))
        in_maps.append({
            "xt_t": xt_dr,
            "x_t": x_dr,
            "cj": cj,
            "bcol": bcol,
            "sgn": sgn_vec,
            "nxsq": np.ascontiguousarray(nxsq_full[sl].reshape(1, IC)),
            "bb": b_arr,
        })

    nc = _build_program(n_pos, b_is_zero=float(b_arr.reshape(-1)[0]) == 0.0)
    res = run_bass_kernel_spmd(nc, in_maps, list(range(NCORES)), trace=TRACE)
    global LAST_RESULTS
    LAST_RESULTS = res

    preds = np.concatenate([res.results[k]["out"][0] for k in range(NCORES)])
    return preds.reshape(N, 1).astype(np.float32)


# revision 30
# speedup vs baseline: 1.1505x; 1.1505x over previous
"""RBF-kernel SVM prediction on 8 Trainium2 NeuronCores.

predictions = exp(-g*||x_i - t_j||^2) @ (alphas*y) + b,  g = 0.5

Strategy (per sharding hint): shard X rows 8-way, replicate train side.
Math is factorized as
    pred_i = exp(-g*||x_i||^2) * sum_j y_j * exp(x_i . t_j + c_j) + b
    c_j    = -g*||t_j||^2 + ln(alpha_j)
so the train-side affine terms ride per-partition biases and the
query-side factor is a per-row epilogue scale. Train points are host-sorted
by label so the +/- y_j signs become whole-tile add/sub.

Per-core engine balance (the loop is exp-throughput bound):
 - PE runs the G = Xt.X^T GEMM in fp8 DoubleRow (2x rate), plus weighted
   ones-matvecs that accumulate ~half the exp'd tiles straight into a
   persistent PSUM row (one accumulation group across the whole loop).
 - ACT does true exp for ~2/3 of tiles (PSUM source, per-partition bias).
 - DVE runs a custom fused op for the rest, computing bf16(e^x) bit
   patterns directly (Schraudolph: bits16 = clamp(A*(G+c)+B, 0, 32000)
   as uint16; the fp32 clamp precedes conversion so no NaN/Inf patterns
   can appear), plus bf16 tensor-tensor accumulates. Exps of DVE-acc'd
   tiles land pairwise in [128, 2048] tiles so one TT covers two j-tiles.
GPSIMD is left idle: it shares an SBUF port with the DVE and measured
as dragging all DVE 2x ops down to 1x when used for accumulates.
"""

import os
import sys

import numpy as np

for _p in ("/opt/trn_rl_repo", "/root/.axon_site/_ro/trn_rl_repo"):
    if os.path.isdir(_p) and _p not in sys.path:
        sys.path.append(_p)

import ml_dtypes

import concourse.bass as bass
import concourse.tile as tile
from concourse import bacc, mybir
from concourse.bass_utils import run_bass_kernel_spmd

GAMMA = 0.5
N, M, D = 8192, 8192, 256
NCORES = 8
IC = N // NCORES          # query rows per core (1024)
JT = M // 128             # j-tiles (64)
F32 = mybir.dt.float32
BF16 = mybir.dt.bfloat16
FP8 = mybir.dt.float8e4
U16 = mybir.dt.uint16
FP32_MIN_NORMAL = 1.1754944e-38

# Schraudolph constants for bf16: bits16(e^x) ~ A*x + B, clamped to [0, CLAMP_HI].
SCH_A = 184.6650784   # 128 / ln(2)
SCH_B = 16250.496     # 128 * (127 - 0.0430)
SCH_CLAMP = 32000.0   # < 0x7F80 (inf); e^x here never exceeds e^10 anyway

N_MAGIC = 20          # tiles exp'd on the DVE
N_DMA_ACC = 0         # tiles accumulated by CCE DMAs (SWDGE too slow; off)
N_WARM = 40           # PE warmup matmuls (HAM clock-gate release)
MV_TAIL = 56          # tiles >= this always accumulate via PE matvec
ACC_LAG = 2           # emit tile t's accumulate after tile t+ACC_LAG's GEMM

# Set by test harness to collect a profile; harness grading leaves it off.
TRACE = False
DEBUG_S = True            # emit the pre-mask s row for test-side validation
LAST_RESULTS = None

_EXP_OP = None


def _register_exp_op():
    """Register the custom DVE op computing bf16 exp bit patterns.

    body = min(max(Src0*C0 + C1, 0), C2), written to a uint16 tile whose
    bits, reinterpreted as bf16, approximate e^(Src0 + c) (C1 carries the
    per-partition c bias pre-scaled by A).
    """
    global _EXP_OP
    if _EXP_OP is not None:
        return _EXP_OP
    from concourse import dve_ops as dvo
    from concourse.dve_ops import DveOp
    from concourse.dve_spec import Spec, Src0, C0, C1, C2, Zero, maxx, minn, lower
    from concourse.dve_uop import DveOpSpec

    name = "EXP_BITS_U16_ANT"
    for op in dvo.OPS:
        if op.name == name:
            _EXP_OP = op
            return op
    spec = Spec(
        body=minn(maxx(Src0 * C0 + C1, Zero), C2),
        reference=lambda in0, in1, s0, s1, imm2: np.minimum(
            np.maximum(in0.astype(np.float32) * s0 + s1, 0.0), imm2
        ),
    )
    opcode = dvo._CUSTOM_DVE_ROW_BASE + len(dvo.OPS)
    shas = {}
    for ver in ("v3", "v4"):
        tmp = DveOpSpec(name=name, opcode=opcode, uops=lower(spec, ver=ver),
                        rd1_en=False)
        shas[ver] = tmp.sha(ver)
    op = DveOp(name, spec, subdim=False, uops_sha=shas)
    dvo.OPS.append(op)
    dvo.CUSTOM_DVE_SPECS[name] = spec
    dvo._SUB_OPCODE_FOR_NAME[name] = opcode
    _EXP_OP = op
    return op


def _plan_tiles(n_pos: int):
    """Per-tile engine assignment.

    Returns (magic, acc_mode) where acc_mode[t] is one of 'mv' (PE
    matvec), 'dve' (DVE TT into accs), 'stt' (mixed-sign tile).
    """
    tb = n_pos // 128 if n_pos % 128 else -1
    # Spread DVE-exp tiles over [4, JT): the first tiles stay on ACT so the
    # pipeline fill isn't gated on the DVE finishing its startup memsets.
    magic = set(4 + int(round(i * (JT - 4) / N_MAGIC)) for i in range(N_MAGIC))
    k = 4
    while len(magic) < N_MAGIC:          # dedupe fallback
        if k not in magic:
            magic.add(k)
        k += 1

    acc_mode = {}
    n_dma = 0
    for t in range(JT):
        if t == tb:
            acc_mode[t] = "stt"
        elif t % 2 == 0 or t >= MV_TAIL:
            acc_mode[t] = "mv"
        elif n_dma < N_DMA_ACC:
            acc_mode[t] = "dma"
            n_dma += 1
        else:
            acc_mode[t] = "dve"
    return magic, acc_mode


def _build_program(n_pos: int, b_is_zero: bool):
    exp_op = _register_exp_op()
    magic, acc_mode = _plan_tiles(n_pos)
    nc = bacc.Bacc()

    # fp8 DoubleRow operands: [ki, ksub, col] with d = ksub*128 + ki.
    xt_t = nc.dram_tensor("xt_t", [128, 2, M], FP8, kind="ExternalInput")
    x_t = nc.dram_tensor("x_t", [128, 2, IC], FP8, kind="ExternalInput")
    cj = nc.dram_tensor("cj", [128, JT], F32, kind="ExternalInput")
    bcol = nc.dram_tensor("bcol", [128, JT], F32, kind="ExternalInput")
    sgn = nc.dram_tensor("sgn", [128, 1], BF16, kind="ExternalInput")
    nxsq = nc.dram_tensor("nxsq", [1, IC], F32, kind="ExternalInput")
    bb = nc.dram_tensor("bb", [1, 1], F32, kind="ExternalInput")
    out = nc.dram_tensor("out", [1, IC], F32, kind="ExternalOutput")
    s_out = nc.dram_tensor("s_out", [1, IC], F32, kind="ExternalOutput")

    NCHUNK = 16           # xt column chunks so matmuls wait on small DMAs
    CW = M // NCHUNK      # 512 j-columns per chunk

    tb = n_pos // 128 if n_pos % 128 else -1   # mixed-sign boundary tile

    def tile_sign(t):
        return +1 if (t + 1) * 128 <= n_pos else -1

    with tile.TileContext(nc) as tc:
        with (
            tc.tile_pool(name="singles", bufs=1) as singles,
            tc.tile_pool(name="epool", bufs=6) as epool,
            tc.tile_pool(name="gpsum", bufs=3, space="PSUM") as gpsum,
            tc.tile_pool(name="spsum", bufs=1, space="PSUM") as spsum,
        ):
            # Resident inputs, all on the sync HWDGE queue (the scalar queue
            # executes on the ACT engine and steals exp throughput), small
            # first-needed operands ahead of the bulk xt chunks.
            x_sb = singles.tile([128, 2, IC], FP8, tag="x")
            nc.sync.dma_start(out=x_sb, in_=x_t[:, :, :])
            cj_sb = singles.tile([128, JT], F32, tag="cj")
            nc.sync.dma_start(out=cj_sb, in_=cj[:, :])
            bcol_sb = singles.tile([128, JT], F32, tag="bcol")
            nc.sync.dma_start(out=bcol_sb, in_=bcol[:, :])
            sgn_sb = singles.tile([128, 1], BF16, tag="sgn")
            nc.sync.dma_start(out=sgn_sb, in_=sgn[:, :])
            nxsq_sb = singles.tile([1, IC], F32, tag="nxsq")
            nc.sync.dma_start(out=nxsq_sb, in_=nxsq[:, :])
            b_sb = singles.tile([1, 1], F32, tag="b")
            nc.sync.dma_start(out=b_sb, in_=bb[:, :])
            xt_sb = [None] * NCHUNK
            for ck in range(NCHUNK):
                t = singles.tile([128, 2, CW], FP8, tag=f"xt_{ck}")
                nc.sync.dma_start(
                    out=t, in_=xt_t[:, :, ck * CW:(ck + 1) * CW],
                )
                xt_sb[ck] = t

            # Warm the PE while input DMAs are in flight (HAM clock gate).
            # warm_w memset is first so warmups aren't gated on other memsets.
            warm_w = singles.tile([128, 128], BF16, tag="warm_w")
            nc.vector.memset(warm_w, 0.0)
            warm_ps = spsum.tile([1, 128], F32, tag="s")
            for _ in range(N_WARM):
                nc.tensor.matmul(
                    out=warm_ps, lhsT=warm_w[:, 0:1], rhs=warm_w[:, :],
                    start=True, stop=True,
                )

            ones_sb = singles.tile([128, 1], BF16, tag="ones")
            nc.vector.memset(ones_sb, 1.0)
            mones_sb = singles.tile([128, 1], BF16, tag="mones")
            nc.vector.memset(mones_sb, -1.0)
            accs = singles.tile([128, IC], BF16, tag="accs")
            nc.vector.memset(accs, 0.0)
            acc_dp = singles.tile([128, IC], BF16, tag="acc_dp")
            nc.vector.memset(acc_dp, 0.0)
            acc_dn = singles.tile([128, IC], BF16, tag="acc_dn")
            nc.vector.memset(acc_dn, 0.0)

            # Query-side factor, computed early so ACT's table load happens
            # during the DMA window.
            e_row = singles.tile([1, IC], F32, tag="e_row")
            nc.scalar.activation(
                out=e_row, in_=nxsq_sb, func=mybir.ActivationFunctionType.Exp
            )
            # Emulate fp32 FTZ on the factor: the reference's direct
            # exp(-g*d) underflows to 0; keep the factored path bit-identical.
            m_row = singles.tile([1, IC], F32, tag="m_row")
            nc.vector.tensor_scalar(
                out=m_row, in0=e_row, scalar1=FP32_MIN_NORMAL, scalar2=None,
                op0=mybir.AluOpType.is_ge,
            )
            nc.vector.tensor_mul(e_row, e_row, m_row)

            # s_ps accumulates PE-matvec'd tiles across the whole j-loop and
            # receives the folds of acc2/accs at the end. Shares the "s"
            # PSUM slot with warm_ps; the first matvec's start=True clears it.
            s_ps = spsum.tile([1, IC], F32, tag="s")

            e_views = {}          # t -> bf16-view AP of tile t's exp output
            first_mv = [True]

            def emit_gemm(t):
                ck, col = t // 4, (t % 4) * 128
                g_ps = gpsum.tile([128, IC], F32, tag="g", name=f"g{t}")
                for ic in range(2):
                    sl = slice(ic * 512, (ic + 1) * 512)
                    nc.tensor.matmul(
                        out=g_ps[:, sl],
                        lhsT=xt_sb[ck][:, :, col:col + 128],
                        rhs=x_sb[:, :, sl],
                        start=True, stop=True,
                        perf_mode=mybir.MatmulPerfMode.DoubleRow,
                    )
                return g_ps

            def emit_exp(t, g_ps):
                dest = epool.tile([128, IC], BF16, tag="e", name=f"e{t}")
                if t in magic:
                    nc.vector._custom_dve(
                        exp_op, out=dest.bitcast(U16), in0=g_ps,
                        s0=SCH_A, s1=bcol_sb[:, t:t + 1], imm2=SCH_CLAMP,
                    )
                else:
                    nc.scalar.activation(
                        out=dest, in_=g_ps,
                        func=mybir.ActivationFunctionType.Exp,
                        bias=cj_sb[:, t:t + 1], scale=1.0,
                    )
                e_views[t] = dest

            def emit_acc(t):
                mode = acc_mode[t]
                e_t = e_views.pop(t)
                if mode == "dve":
                    if tile_sign(t) > 0:
                        nc.vector.tensor_add(accs, accs, e_t)
                    else:
                        nc.vector.tensor_sub(accs, accs, e_t)
                elif mode == "dma":
                    # CCE read-modify-write accumulate in the DMA engine;
                    # gpsimd only generates descriptors.
                    dst = acc_dp if tile_sign(t) > 0 else acc_dn
                    nc.gpsimd.dma_start(
                        out=dst, in_=e_t, accum_op=mybir.AluOpType.add,
                    )
                elif mode == "stt":
                    nc.vector.scalar_tensor_tensor(
                        out=accs, in0=e_t, scalar=sgn_sb[:, 0:1], in1=accs,
                        op0=mybir.AluOpType.mult, op1=mybir.AluOpType.add,
                    )
                else:
                    w = ones_sb if tile_sign(t) > 0 else mones_sb
                    for ic in range(2):
                        sl = slice(ic * 512, (ic + 1) * 512)
                        nc.tensor.matmul(
                            out=s_ps[:, sl], lhsT=w, rhs=e_t[:, sl],
                            start=first_mv[0], stop=False,
                            skip_group_check=True,
                        )
                    first_mv[0] = False

            for t in range(JT):
                g_ps = emit_gemm(t)
                emit_exp(t, g_ps)
                if t >= ACC_LAG:
                    emit_acc(t - ACC_LAG)
            for t in range(JT - ACC_LAG, JT):
                emit_acc(t)

            # Fold the DVE/DMA accumulators in: s[0,i] += sum_p acc[p,i];
            # the last matmul stops the PSUM accumulation group.
            folds = [(accs, ones_sb), (acc_dp, ones_sb), (acc_dn, mones_sb)]
            for fi, (f, w) in enumerate(folds):
                for ic in range(2):
                    sl = slice(ic * 512, (ic + 1) * 512)
                    last = fi == len(folds) - 1 and ic == 1
                    nc.tensor.matmul(
                        out=s_ps[:, sl], lhsT=w, rhs=f[:, sl],
                        start=first_mv[0], stop=last,
                        skip_group_check=True,
                    )
                    first_mv[0] = False
            if DEBUG_S:
                s_sb = singles.tile([1, IC], F32, tag="s_sb")
                nc.vector.tensor_copy(s_sb, s_ps)
                nc.sync.dma_start(out=s_out[:, :], in_=s_sb)  # pre-mask s
            p_row = singles.tile([1, IC], F32, tag="p_row")
            nc.vector.tensor_mul(p_row, s_ps, e_row)
            if not b_is_zero:
                nc.vector.tensor_scalar(
                    out=p_row, in0=p_row, scalar1=b_sb[0:1, 0:1], scalar2=None,
                    op0=mybir.AluOpType.add,
                )
            nc.sync.dma_start(out=out[:, :], in_=p_row)

    nc.finalize()
    return nc


def kernel(X, X_train, alphas, y_train, b):
    X = np.ascontiguousarray(np.asarray(X, dtype=np.float32))
    X_train = np.ascontiguousarray(np.asarray(X_train, dtype=np.float32))
    alphas = np.asarray(alphas, dtype=np.float32).reshape(M)
    y_train = np.asarray(y_train, dtype=np.float32).reshape(M)
    b_arr = np.asarray(b, dtype=np.float32).reshape(1, 1)

    # Sort train points by label (+1 first), then by c within each label so
    # c values on a partition row are close (enables shared-bias tricks).
    c_all = (-GAMMA * (X_train * X_train).sum(1)
             + np.log(np.maximum(alphas, np.float32(1e-38)))).astype(np.float32)
    perm = np.lexsort((c_all, -y_train))
    n_pos = int((y_train > 0).sum())
    Xt_p = X_train[perm]
    c = c_all[perm]

    cj = np.ascontiguousarray(c.reshape(JT, 128).T)          # [128, JT]
    bcol = (SCH_A * cj + np.float32(SCH_B)).astype(np.float32)
    r = n_pos % 128
    sgn_vec = np.where(np.arange(128) < r, 1.0, -1.0).astype(
        ml_dtypes.bfloat16).reshape(128, 1)

    # fp8 DoubleRow layouts: [ki, ksub, col], d = ksub*128 + ki.
    f8 = ml_dtypes.float8_e4m3fn
    xt_dr = np.ascontiguousarray(
        Xt_p.T.reshape(2, 128, M).transpose(1, 0, 2).astype(f8))
    nxsq_full = (-GAMMA * (X * X).sum(1)).astype(np.float32)

    in_maps = []
    for k in range(NCORES):
        sl = slice(k * IC, (k + 1) * IC)
        x_dr = np.ascontiguousarray(
            X[sl].T.reshape(2, 128, IC).transpose(1, 0, 2).astype(f8))
        in_maps.append({
            "xt_t": xt_dr,
            "x_t": x_dr,
            "cj": cj,
            "bcol": bcol,
            "sgn": sgn_vec,
            "nxsq": np.ascontiguousarray(nxsq_full[sl].reshape(1, IC)),
            "bb": b_arr,
        })

    nc = _build_program(n_pos, b_is_zero=float(b_arr.reshape(-1)[0]) == 0.0)
    res = run_bass_kernel_spmd(nc, in_maps, list(range(NCORES)), trace=TRACE)
    global LAST_RESULTS
    LAST_RESULTS = res

    preds = np.concatenate([res.results[k]["out"][0] for k in range(NCORES)])
    return preds.reshape(N, 1).astype(np.float32)


# revision 36
# speedup vs baseline: 1.1642x; 1.0119x over previous
"""RBF-kernel SVM prediction on 8 Trainium2 NeuronCores.

predictions = exp(-g*||x_i - t_j||^2) @ (alphas*y) + b,  g = 0.5

Strategy (per sharding hint): shard X rows 8-way, replicate train side.
Math is factorized as
    pred_i = exp(-g*||x_i||^2) * sum_j y_j * exp(x_i . t_j + c_j) + b
    c_j    = -g*||t_j||^2 + ln(alpha_j)
so the train-side affine terms ride per-partition biases and the
query-side factor is a per-row epilogue scale. Train points are host-sorted
by label so the +/- y_j signs become whole-tile add/sub.

Per-core engine balance (the loop is exp-throughput bound):
 - PE runs the G = Xt.X^T GEMM in fp8 DoubleRow (2x rate), plus weighted
   ones-matvecs that accumulate ~half the exp'd tiles straight into a
   persistent PSUM row (one accumulation group across the whole loop).
 - ACT does true exp for ~2/3 of tiles (PSUM source, per-partition bias).
 - DVE runs a custom fused op for the rest, computing bf16(e^x) bit
   patterns directly (Schraudolph: bits16 = clamp(A*(G+c)+B, 0, 32000)
   as uint16; the fp32 clamp precedes conversion so no NaN/Inf patterns
   can appear), plus bf16 tensor-tensor accumulates. Exps of DVE-acc'd
   tiles land pairwise in [128, 2048] tiles so one TT covers two j-tiles.
GPSIMD is left idle: it shares an SBUF port with the DVE and measured
as dragging all DVE 2x ops down to 1x when used for accumulates.
"""

import os
import sys

import numpy as np

for _p in ("/opt/trn_rl_repo", "/root/.axon_site/_ro/trn_rl_repo"):
    if os.path.isdir(_p) and _p not in sys.path:
        sys.path.append(_p)

import ml_dtypes

import concourse.bass as bass
import concourse.tile as tile
from concourse import bacc, mybir
from concourse.bass_utils import run_bass_kernel_spmd

GAMMA = 0.5
N, M, D = 8192, 8192, 256
NCORES = 8
IC = N // NCORES          # query rows per core (1024)
JT = M // 128             # j-tiles (64)
F32 = mybir.dt.float32
BF16 = mybir.dt.bfloat16
FP8 = mybir.dt.float8e4
U16 = mybir.dt.uint16
FP32_MIN_NORMAL = 1.1754944e-38

# Schraudolph constants for bf16: bits16(e^x) ~ A*x + B, clamped to [0, CLAMP_HI].
SCH_A = 184.6650784   # 128 / ln(2)
SCH_B = 16250.496     # 128 * (127 - 0.0430)
SCH_CLAMP = 32000.0   # < 0x7F80 (inf); e^x here never exceeds e^10 anyway

def _env(name, default):
    return int(os.environ.get(name, default))


N_MAGIC = _env("KN_MAGIC", 22)    # tiles exp'd on the DVE
N_DMA_ACC = 0         # tiles accumulated by CCE DMAs (SWDGE too slow; off)
N_WARM = _env("KN_WARM", 40)      # PE warmup matmuls (HAM clock-gate release)
MV_TAIL = _env("KN_MVTAIL", 56)   # tiles >= this always accumulate via PE matvec
ACC_LAG = _env("KN_LAG", 2)       # emit tile t's acc after tile t+ACC_LAG's GEMM
EPOOL = _env("KN_EPOOL", 6)       # e-tile pool buffers

# Set by test harness to collect a profile; harness grading leaves it off.
TRACE = False
DEBUG_S = True            # emit the pre-mask s row for test-side validation
LAST_RESULTS = None

_EXP_OP = None


def _register_exp_op():
    """Register the custom DVE op computing bf16 exp bit patterns.

    body = min(max(Src0*C0 + C1, 0), C2), written to a uint16 tile whose
    bits, reinterpreted as bf16, approximate e^(Src0 + c) (C1 carries the
    per-partition c bias pre-scaled by A).
    """
    global _EXP_OP
    if _EXP_OP is not None:
        return _EXP_OP
    from concourse import dve_ops as dvo
    from concourse.dve_ops import DveOp
    from concourse.dve_spec import Spec, Src0, C0, C1, C2, Zero, maxx, minn, lower
    from concourse.dve_uop import DveOpSpec

    name = "EXP_BITS_U16_ANT"
    for op in dvo.OPS:
        if op.name == name:
            _EXP_OP = op
            return op
    spec = Spec(
        body=minn(maxx(Src0 * C0 + C1, Zero), C2),
        reference=lambda in0, in1, s0, s1, imm2: np.minimum(
            np.maximum(in0.astype(np.float32) * s0 + s1, 0.0), imm2
        ),
    )
    opcode = dvo._CUSTOM_DVE_ROW_BASE + len(dvo.OPS)
    shas = {}
    for ver in ("v3", "v4"):
        tmp = DveOpSpec(name=name, opcode=opcode, uops=lower(spec, ver=ver),
                        rd1_en=False)
        shas[ver] = tmp.sha(ver)
    op = DveOp(name, spec, subdim=False, uops_sha=shas)
    dvo.OPS.append(op)
    dvo.CUSTOM_DVE_SPECS[name] = spec
    dvo._SUB_OPCODE_FOR_NAME[name] = opcode
    _EXP_OP = op
    return op


def _plan_tiles(n_pos: int):
    """Per-tile engine assignment.

    Returns (magic, acc_mode) where acc_mode[t] is one of 'mv' (PE
    matvec), 'dve' (DVE TT into accs), 'stt' (mixed-sign tile).
    """
    tb = n_pos // 128 if n_pos % 128 else -1
    # Spread DVE-exp tiles over [4, JT): the first tiles stay on ACT so the
    # pipeline fill isn't gated on the DVE finishing its startup memsets.
    magic = set(4 + int(round(i * (JT - 4) / N_MAGIC)) for i in range(N_MAGIC))
    k = 4
    while len(magic) < N_MAGIC:          # dedupe fallback
        if k not in magic:
            magic.add(k)
        k += 1

    acc_mode = {}
    n_dma = 0
    for t in range(JT):
        if t == tb:
            acc_mode[t] = "stt"
        elif t % 2 == 0 or t >= MV_TAIL:
            acc_mode[t] = "mv"
        elif n_dma < N_DMA_ACC:
            acc_mode[t] = "dma"
            n_dma += 1
        else:
            acc_mode[t] = "dve"
    return magic, acc_mode


def _build_program(n_pos: int, b_is_zero: bool):
    exp_op = _register_exp_op()
    magic, acc_mode = _plan_tiles(n_pos)
    nc = bacc.Bacc()

    # fp8 DoubleRow operands: [ki, ksub, col] with d = ksub*128 + ki.
    xt_t = nc.dram_tensor("xt_t", [128, 2, M], FP8, kind="ExternalInput")
    x_t = nc.dram_tensor("x_t", [128, 2, IC], FP8, kind="ExternalInput")
    cj = nc.dram_tensor("cj", [128, JT], F32, kind="ExternalInput")
    bcol = nc.dram_tensor("bcol", [128, JT], F32, kind="ExternalInput")
    sgn = nc.dram_tensor("sgn", [128, 1], BF16, kind="ExternalInput")
    nxsq = nc.dram_tensor("nxsq", [1, IC], F32, kind="ExternalInput")
    bb = nc.dram_tensor("bb", [1, 1], F32, kind="ExternalInput")
    out = nc.dram_tensor("out", [1, IC], F32, kind="ExternalOutput")
    s_out = nc.dram_tensor("s_out", [1, IC], F32, kind="ExternalOutput")

    NCHUNK = 16           # xt column chunks so matmuls wait on small DMAs
    CW = M // NCHUNK      # 512 j-columns per chunk

    tb = n_pos // 128 if n_pos % 128 else -1   # mixed-sign boundary tile

    def tile_sign(t):
        return +1 if (t + 1) * 128 <= n_pos else -1

    with tile.TileContext(nc) as tc:
        with (
            tc.tile_pool(name="singles", bufs=1) as singles,
            tc.tile_pool(name="epool", bufs=EPOOL) as epool,
            tc.tile_pool(name="gpsum", bufs=3, space="PSUM") as gpsum,
            tc.tile_pool(name="spsum", bufs=1, space="PSUM") as spsum,
        ):
            # Resident inputs, all on the sync HWDGE queue (the scalar queue
            # executes on the ACT engine and steals exp throughput), small
            # first-needed operands ahead of the bulk xt chunks.
            x_sb = singles.tile([128, 2, IC], FP8, tag="x")
            nc.sync.dma_start(out=x_sb, in_=x_t[:, :, :])
            cj_sb = singles.tile([128, JT], F32, tag="cj")
            nc.sync.dma_start(out=cj_sb, in_=cj[:, :])
            bcol_sb = singles.tile([128, JT], F32, tag="bcol")
            nc.sync.dma_start(out=bcol_sb, in_=bcol[:, :])
            sgn_sb = singles.tile([128, 1], BF16, tag="sgn")
            nc.sync.dma_start(out=sgn_sb, in_=sgn[:, :])
            nxsq_sb = singles.tile([1, IC], F32, tag="nxsq")
            nc.sync.dma_start(out=nxsq_sb, in_=nxsq[:, :])
            b_sb = singles.tile([1, 1], F32, tag="b")
            nc.sync.dma_start(out=b_sb, in_=bb[:, :])
            xt_sb = [None] * NCHUNK
            for ck in range(NCHUNK):
                t = singles.tile([128, 2, CW], FP8, tag=f"xt_{ck}")
                nc.sync.dma_start(
                    out=t, in_=xt_t[:, :, ck * CW:(ck + 1) * CW],
                )
                xt_sb[ck] = t

            # Warm the PE while input DMAs are in flight (HAM clock gate).
            # warm_w memset is first so warmups aren't gated on other memsets.
            warm_w = singles.tile([128, 128], BF16, tag="warm_w")
            nc.vector.memset(warm_w, 0.0)
            warm_ps = spsum.tile([1, 128], F32, tag="s")
            for _ in range(N_WARM):
                nc.tensor.matmul(
                    out=warm_ps, lhsT=warm_w[:, 0:1], rhs=warm_w[:, :],
                    start=True, stop=True,
                )

            ones_sb = singles.tile([128, 1], BF16, tag="ones")
            nc.vector.memset(ones_sb, 1.0)
            mones_sb = singles.tile([128, 1], BF16, tag="mones")
            nc.vector.memset(mones_sb, -1.0)
            accs = singles.tile([128, IC], BF16, tag="accs")
            nc.vector.memset(accs, 0.0)

            # Query-side factor, computed early so ACT's table load happens
            # during the DMA window.
            e_row = singles.tile([1, IC], F32, tag="e_row")
            nc.scalar.activation(
                out=e_row, in_=nxsq_sb, func=mybir.ActivationFunctionType.Exp
            )
            # Emulate fp32 FTZ on the factor: the reference's direct
            # exp(-g*d) underflows to 0; keep the factored path bit-identical.
            m_row = singles.tile([1, IC], F32, tag="m_row")
            nc.vector.tensor_scalar(
                out=m_row, in0=e_row, scalar1=FP32_MIN_NORMAL, scalar2=None,
                op0=mybir.AluOpType.is_ge,
            )
            nc.vector.tensor_mul(e_row, e_row, m_row)

            # s_ps accumulates PE-matvec'd tiles across the whole j-loop and
            # receives the folds of acc2/accs at the end. Shares the "s"
            # PSUM slot with warm_ps; the first matvec's start=True clears it.
            s_ps = spsum.tile([1, IC], F32, tag="s")

            e_views = {}          # t -> bf16-view AP of tile t's exp output
            first_mv = [True]

            def emit_gemm(t):
                ck, col = t // 4, (t % 4) * 128
                g_ps = gpsum.tile([128, IC], F32, tag="g", name=f"g{t}")
                for ic in range(2):
                    sl = slice(ic * 512, (ic + 1) * 512)
                    nc.tensor.matmul(
                        out=g_ps[:, sl],
                        lhsT=xt_sb[ck][:, :, col:col + 128],
                        rhs=x_sb[:, :, sl],
                        start=True, stop=True,
                        perf_mode=mybir.MatmulPerfMode.DoubleRow,
                    )
                return g_ps

            def emit_exp(t, g_ps):
                dest = epool.tile([128, IC], BF16, tag="e", name=f"e{t}")
                if t in magic:
                    nc.vector._custom_dve(
                        exp_op, out=dest.bitcast(U16), in0=g_ps,
                        s0=SCH_A, s1=bcol_sb[:, t:t + 1], imm2=SCH_CLAMP,
                    )
                else:
                    nc.scalar.activation(
                        out=dest, in_=g_ps,
                        func=mybir.ActivationFunctionType.Exp,
                        bias=cj_sb[:, t:t + 1], scale=1.0,
                    )
                e_views[t] = dest

            def emit_acc(t):
                mode = acc_mode[t]
                e_t = e_views.pop(t)
                if mode == "dve":
                    if tile_sign(t) > 0:
                        nc.vector.tensor_add(accs, accs, e_t)
                    else:
                        nc.vector.tensor_sub(accs, accs, e_t)
                elif mode == "stt":
                    nc.vector.scalar_tensor_tensor(
                        out=accs, in0=e_t, scalar=sgn_sb[:, 0:1], in1=accs,
                        op0=mybir.AluOpType.mult, op1=mybir.AluOpType.add,
                    )
                else:
                    w = ones_sb if tile_sign(t) > 0 else mones_sb
                    for ic in range(2):
                        sl = slice(ic * 512, (ic + 1) * 512)
                        nc.tensor.matmul(
                            out=s_ps[:, sl], lhsT=w, rhs=e_t[:, sl],
                            start=first_mv[0], stop=False,
                            skip_group_check=True,
                        )
                    first_mv[0] = False

            for t in range(JT):
                g_ps = emit_gemm(t)
                emit_exp(t, g_ps)
                if t >= ACC_LAG:
                    emit_acc(t - ACC_LAG)
            for t in range(JT - ACC_LAG, JT):
                emit_acc(t)

            # Fold the DVE accumulator in: s[0,i] += sum_p accs[p,i]; the
            # last matmul stops the PSUM accumulation group.
            for ic in range(2):
                sl = slice(ic * 512, (ic + 1) * 512)
                nc.tensor.matmul(
                    out=s_ps[:, sl], lhsT=ones_sb, rhs=accs[:, sl],
                    start=first_mv[0], stop=ic == 1,
                    skip_group_check=True,
                )
                first_mv[0] = False
            if DEBUG_S:
                s_sb = singles.tile([1, IC], F32, tag="s_sb")
                nc.vector.tensor_copy(s_sb, s_ps)
                nc.sync.dma_start(out=s_out[:, :], in_=s_sb)  # pre-mask s
            p_row = singles.tile([1, IC], F32, tag="p_row")
            nc.vector.tensor_mul(p_row, s_ps, e_row)
            if not b_is_zero:
                nc.vector.tensor_scalar(
                    out=p_row, in0=p_row, scalar1=b_sb[0:1, 0:1], scalar2=None,
                    op0=mybir.AluOpType.add,
                )
            nc.sync.dma_start(out=out[:, :], in_=p_row)

    nc.finalize()
    return nc


def kernel(X, X_train, alphas, y_train, b):
    X = np.ascontiguousarray(np.asarray(X, dtype=np.float32))
    X_train = np.ascontiguousarray(np.asarray(X_train, dtype=np.float32))
    alphas = np.asarray(alphas, dtype=np.float32).reshape(M)
    y_train = np.asarray(y_train, dtype=np.float32).reshape(M)
    b_arr = np.asarray(b, dtype=np.float32).reshape(1, 1)

    # Sort train points by label (+1 first), then by c within each label so
    # c values on a partition row are close (enables shared-bias tricks).
    c_all = (-GAMMA * (X_train * X_train).sum(1)
             + np.log(np.maximum(alphas, np.float32(1e-38)))).astype(np.float32)
    perm = np.lexsort((c_all, -y_train))
    n_pos = int((y_train > 0).sum())
    Xt_p = X_train[perm]
    c = c_all[perm]

    cj = np.ascontiguousarray(c.reshape(JT, 128).T)          # [128, JT]
    bcol = (SCH_A * cj + np.float32(SCH_B)).astype(np.float32)
    r = n_pos % 128
    sgn_vec = np.where(np.arange(128) < r, 1.0, -1.0).astype(
        ml_dtypes.bfloat16).reshape(128, 1)

    # fp8 DoubleRow layouts: [ki, ksub, col], d = ksub*128 + ki.
    f8 = ml_dtypes.float8_e4m3fn
    xt_dr = np.ascontiguousarray(
        Xt_p.T.reshape(2, 128, M).transpose(1, 0, 2).astype(f8))
    nxsq_full = (-GAMMA * (X * X).sum(1)).astype(np.float32)

    in_maps = []
    for k in range(NCORES):
        sl = slice(k * IC, (k + 1) * IC)
        x_dr = np.ascontiguousarray(
            X[sl].T.reshape(2, 128, IC).transpose(1, 0, 2).astype(f8))
        in_maps.append({
            "xt_t": xt_dr,
            "x_t": x_dr,
            "cj": cj,
            "bcol": bcol,
            "sgn": sgn_vec,
            "nxsq": np.ascontiguousarray(nxsq_full[sl].reshape(1, IC)),
            "bb": b_arr,
        })

    nc = _build_program(n_pos, b_is_zero=float(b_arr.reshape(-1)[0]) == 0.0)
    res = run_bass_kernel_spmd(nc, in_maps, list(range(NCORES)), trace=TRACE)
    global LAST_RESULTS
    LAST_RESULTS = res

    preds = np.concatenate([res.results[k]["out"][0] for k in range(NCORES)])
    return preds.reshape(N, 1).astype(np.float32)


# revision 44
# speedup vs baseline: 1.1711x; 1.0059x over previous
"""RBF-kernel SVM prediction on 8 Trainium2 NeuronCores.

predictions = exp(-g*||x_i - t_j||^2) @ (alphas*y) + b,  g = 0.5

Strategy (per sharding hint): shard X rows 8-way, replicate train side.
Math is factorized as
    pred_i = exp(-g*||x_i||^2) * sum_j y_j * exp(x_i . t_j + c_j) + b
    c_j    = -g*||t_j||^2 + ln(alpha_j)
so the train-side affine terms ride per-partition biases and the
query-side factor is a per-row epilogue scale. Train points are host-sorted
by label so the +/- y_j signs become whole-tile add/sub.

Per-core engine balance (the loop is exp-throughput bound):
 - PE runs the G = Xt.X^T GEMM in fp8 DoubleRow (2x rate), plus weighted
   ones-matvecs that accumulate ~half the exp'd tiles straight into a
   persistent PSUM row (one accumulation group across the whole loop).
 - ACT does true exp for ~2/3 of tiles (PSUM source, per-partition bias).
 - DVE runs a custom fused op for the rest, computing bf16(e^x) bit
   patterns directly (Schraudolph: bits16 = clamp(A*(G+c)+B, 0, 32000)
   as uint16; the fp32 clamp precedes conversion so no NaN/Inf patterns
   can appear), plus bf16 tensor-tensor accumulates. Exps of DVE-acc'd
   tiles land pairwise in [128, 2048] tiles so one TT covers two j-tiles.
GPSIMD is left idle: it shares an SBUF port with the DVE and measured
as dragging all DVE 2x ops down to 1x when used for accumulates.
"""

import os
import sys

import numpy as np

for _p in ("/opt/trn_rl_repo", "/root/.axon_site/_ro/trn_rl_repo"):
    if os.path.isdir(_p) and _p not in sys.path:
        sys.path.append(_p)

import ml_dtypes

import concourse.bass as bass
import concourse.tile as tile
from concourse import bacc, mybir
from concourse.bass_utils import run_bass_kernel_spmd

GAMMA = 0.5
N, M, D = 8192, 8192, 256
NCORES = 8
IC = N // NCORES          # query rows per core (1024)
JT = M // 128             # j-tiles (64)
F32 = mybir.dt.float32
BF16 = mybir.dt.bfloat16
FP8 = mybir.dt.float8e4
U16 = mybir.dt.uint16
FP32_MIN_NORMAL = 1.1754944e-38

# Schraudolph constants for bf16: bits16(e^x) ~ A*x + B, clamped to [0, CLAMP_HI].
SCH_A = 184.6650784   # 128 / ln(2)
SCH_B = 16250.496     # 128 * (127 - 0.0430)
SCH_CLAMP = 32000.0   # < 0x7F80 (inf); e^x here never exceeds e^10 anyway

def _env(name, default):
    return int(os.environ.get(name, default))


N_MAGIC = _env("KN_MAGIC", 22)    # tiles exp'd on the DVE
N_DMA_GRP = _env("KN_DMAGRP", 0)  # groups of 4 tiles accumulated by CCE DMAs
N_WARM = _env("KN_WARM", 40)      # PE warmup matmuls (HAM clock-gate release)
MV_TAIL = _env("KN_MVTAIL", 56)   # tiles >= this always accumulate via PE matvec
ACC_LAG = _env("KN_LAG", 2)       # emit tile t's acc after tile t+ACC_LAG's GEMM
EPOOL = _env("KN_EPOOL", 6)       # e-tile pool buffers

# Set by test harness to collect a profile; harness grading leaves it off.
TRACE = False
DEBUG_S = True            # emit the pre-mask s row for test-side validation
LAST_RESULTS = None

_EXP_OP = None


def _register_exp_op():
    """Register the custom DVE op computing bf16 exp bit patterns.

    body = min(max(Src0*C0 + C1, 0), C2), written to a uint16 tile whose
    bits, reinterpreted as bf16, approximate e^(Src0 + c) (C1 carries the
    per-partition c bias pre-scaled by A).
    """
    global _EXP_OP
    if _EXP_OP is not None:
        return _EXP_OP
    from concourse import dve_ops as dvo
    from concourse.dve_ops import DveOp
    from concourse.dve_spec import Spec, Src0, C0, C1, C2, Zero, maxx, minn, lower
    from concourse.dve_uop import DveOpSpec

    name = "EXP_BITS_U16_ANT"
    for op in dvo.OPS:
        if op.name == name:
            _EXP_OP = op
            return op
    spec = Spec(
        body=minn(maxx(Src0 * C0 + C1, Zero), C2),
        reference=lambda in0, in1, s0, s1, imm2: np.minimum(
            np.maximum(in0.astype(np.float32) * s0 + s1, 0.0), imm2
        ),
    )
    opcode = dvo._CUSTOM_DVE_ROW_BASE + len(dvo.OPS)
    shas = {}
    for ver in ("v3", "v4"):
        tmp = DveOpSpec(name=name, opcode=opcode, uops=lower(spec, ver=ver),
                        rd1_en=False)
        shas[ver] = tmp.sha(ver)
    op = DveOp(name, spec, subdim=False, uops_sha=shas)
    dvo.OPS.append(op)
    dvo.CUSTOM_DVE_SPECS[name] = spec
    dvo._SUB_OPCODE_FOR_NAME[name] = opcode
    _EXP_OP = op
    return op


def _plan_tiles(n_pos: int):
    """Per-tile engine assignment.

    Returns (magic, acc_mode) where acc_mode[t] is one of 'mv' (PE
    matvec), 'dve' (DVE TT into accs), 'stt' (mixed-sign tile).
    """
    tb = n_pos // 128 if n_pos % 128 else -1
    # Spread DVE-exp tiles over [4, JT): the first tiles stay on ACT so the
    # pipeline fill isn't gated on the DVE finishing its startup memsets.
    magic = set(4 + int(round(i * (JT - 4) / N_MAGIC)) for i in range(N_MAGIC))
    k = 4
    while len(magic) < N_MAGIC:          # dedupe fallback
        if k not in magic:
            magic.add(k)
        k += 1

    acc_mode = {}
    for t in range(JT):
        if t == tb:
            acc_mode[t] = "stt"
        elif t % 2 == 0 or t >= MV_TAIL:
            acc_mode[t] = "mv"
        else:
            acc_mode[t] = "dve"

    # Form groups of 4 same-sign 'dve' tiles for batched CCE-DMA accumulate.
    dve_tiles = [t for t in range(JT) if acc_mode[t] == "dve"]
    groups = []
    cur = []
    for t in dve_tiles:
        if len(groups) >= N_DMA_GRP:
            break
        sign = +1 if (t + 1) * 128 <= n_pos else -1
        if cur and sign != cur[0][1]:
            cur = []
        cur.append((t, sign))
        if len(cur) == 4:
            groups.append([x[0] for x in cur])
            for x in cur:
                acc_mode[x[0]] = "grp"
            cur = []
    return magic, acc_mode, groups


def _build_program(n_pos: int, b_is_zero: bool):
    exp_op = _register_exp_op()
    magic, acc_mode, groups = _plan_tiles(n_pos)
    grp_of = {}
    for gi, g in enumerate(groups):
        for slot, t in enumerate(g):
            grp_of[t] = (gi, slot)
    nc = bacc.Bacc()

    # fp8 DoubleRow operands: [ki, ksub, col] with d = ksub*128 + ki.
    xt_t = nc.dram_tensor("xt_t", [128, 2, M], FP8, kind="ExternalInput")
    x_t = nc.dram_tensor("x_t", [128, 2, IC], FP8, kind="ExternalInput")
    cj = nc.dram_tensor("cj", [128, JT], F32, kind="ExternalInput")
    bcol = nc.dram_tensor("bcol", [128, JT], F32, kind="ExternalInput")
    sgn = nc.dram_tensor("sgn", [128, 1], BF16, kind="ExternalInput")
    nxsq = nc.dram_tensor("nxsq", [1, IC], F32, kind="ExternalInput")
    bb = nc.dram_tensor("bb", [1, 1], F32, kind="ExternalInput")
    out = nc.dram_tensor("out", [1, IC], F32, kind="ExternalOutput")
    s_out = nc.dram_tensor("s_out", [1, IC], F32, kind="ExternalOutput")

    NCHUNK = 16           # xt column chunks so matmuls wait on small DMAs
    CW = M // NCHUNK      # 512 j-columns per chunk

    tb = n_pos // 128 if n_pos % 128 else -1   # mixed-sign boundary tile

    def tile_sign(t):
        return +1 if (t + 1) * 128 <= n_pos else -1

    with tile.TileContext(nc) as tc:
        with (
            tc.tile_pool(name="singles", bufs=1) as singles,
            tc.tile_pool(name="epool", bufs=EPOOL) as epool,
            tc.tile_pool(name="gpsum", bufs=3, space="PSUM") as gpsum,
            tc.tile_pool(name="spsum", bufs=1, space="PSUM") as spsum,
        ):
            # Resident inputs, all on the sync HWDGE queue (the scalar queue
            # executes on the ACT engine and steals exp throughput), small
            # first-needed operands ahead of the bulk xt chunks.
            x_sb = singles.tile([128, 2, IC], FP8, tag="x")
            nc.sync.dma_start(out=x_sb, in_=x_t[:, :, :])
            cj_sb = singles.tile([128, JT], F32, tag="cj")
            nc.sync.dma_start(out=cj_sb, in_=cj[:, :])
            bcol_sb = singles.tile([128, JT], F32, tag="bcol")
            nc.sync.dma_start(out=bcol_sb, in_=bcol[:, :])
            sgn_sb = singles.tile([128, 1], BF16, tag="sgn")
            nc.sync.dma_start(out=sgn_sb, in_=sgn[:, :])
            nxsq_sb = singles.tile([1, IC], F32, tag="nxsq")
            nc.sync.dma_start(out=nxsq_sb, in_=nxsq[:, :])
            b_sb = singles.tile([1, 1], F32, tag="b")
            nc.sync.dma_start(out=b_sb, in_=bb[:, :])
            xt_sb = [None] * NCHUNK
            for ck in range(NCHUNK):
                t = singles.tile([128, 2, CW], FP8, tag=f"xt_{ck}")
                nc.sync.dma_start(
                    out=t, in_=xt_t[:, :, ck * CW:(ck + 1) * CW],
                )
                xt_sb[ck] = t

            # Warm the PE while input DMAs are in flight (HAM clock gate).
            # warm_w memset is first so warmups aren't gated on other memsets.
            warm_w = singles.tile([128, 128], BF16, tag="warm_w")
            nc.vector.memset(warm_w, 0.0)
            warm_ps = spsum.tile([1, 128], F32, tag="s")
            for _ in range(N_WARM):
                nc.tensor.matmul(
                    out=warm_ps, lhsT=warm_w[:, 0:1], rhs=warm_w[:, :],
                    start=True, stop=True,
                )

            ones_sb = singles.tile([128, 1], BF16, tag="ones")
            nc.vector.memset(ones_sb, 1.0)
            mones_sb = singles.tile([128, 1], BF16, tag="mones")
            nc.vector.memset(mones_sb, -1.0)
            accs = singles.tile([128, IC], BF16, tag="accs")
            nc.vector.memset(accs, 0.0)
            if groups:
                acc_dp = singles.tile([128, IC], BF16, tag="acc_dp")
                nc.vector.memset(acc_dp, 0.0)
                acc_dn = singles.tile([128, IC], BF16, tag="acc_dn")
                nc.vector.memset(acc_dn, 0.0)

            # Query-side factor, computed early so ACT's table load happens
            # during the DMA window.
            e_row = singles.tile([1, IC], F32, tag="e_row")
            nc.scalar.activation(
                out=e_row, in_=nxsq_sb, func=mybir.ActivationFunctionType.Exp
            )
            # Emulate fp32 FTZ on the factor: the reference's direct
            # exp(-g*d) underflows to 0; keep the factored path bit-identical.
            m_row = singles.tile([1, IC], F32, tag="m_row")
            nc.vector.tensor_scalar(
                out=m_row, in0=e_row, scalar1=FP32_MIN_NORMAL, scalar2=None,
                op0=mybir.AluOpType.is_ge,
            )
            nc.vector.tensor_mul(e_row, e_row, m_row)

            # s_ps accumulates PE-matvec'd tiles across the whole j-loop and
            # receives the folds of acc2/accs at the end. Shares the "s"
            # PSUM slot with warm_ps; the first matvec's start=True clears it.
            s_ps = spsum.tile([1, IC], F32, tag="s")

            e_views = {}          # t -> bf16-view AP of tile t's exp output
            e4_tiles = {}         # group idx -> wide [128, 4, IC] tile
            first_mv = [True]

            def emit_gemm(t):
                ck, col = t // 4, (t % 4) * 128
                g_ps = gpsum.tile([128, IC], F32, tag="g", name=f"g{t}")
                for ic in range(2):
                    sl = slice(ic * 512, (ic + 1) * 512)
                    nc.tensor.matmul(
                        out=g_ps[:, sl],
                        lhsT=xt_sb[ck][:, :, col:col + 128],
                        rhs=x_sb[:, :, sl],
                        start=True, stop=True,
                        perf_mode=mybir.MatmulPerfMode.DoubleRow,
                    )
                return g_ps

            def emit_exp(t, g_ps):
                if t in grp_of:
                    gi, slot = grp_of[t]
                    if gi not in e4_tiles:
                        e4_tiles[gi] = epool.tile(
                            [128, 4, IC], BF16, tag="e4", bufs=2, name=f"e4_{gi}")
                    dest = e4_tiles[gi][:, slot, :]
                else:
                    dest = epool.tile([128, IC], BF16, tag="e", name=f"e{t}")
                if t in magic:
                    nc.vector._custom_dve(
                        exp_op, out=dest.bitcast(U16), in0=g_ps,
                        s0=SCH_A, s1=bcol_sb[:, t:t + 1], imm2=SCH_CLAMP,
                    )
                else:
                    nc.scalar.activation(
                        out=dest, in_=g_ps,
                        func=mybir.ActivationFunctionType.Exp,
                        bias=cj_sb[:, t:t + 1], scale=1.0,
                    )
                e_views[t] = dest

            def emit_acc(t):
                mode = acc_mode[t]
                e_t = e_views.pop(t)
                if mode == "dve":
                    if tile_sign(t) > 0:
                        nc.vector.tensor_add(accs, accs, e_t)
                    else:
                        nc.vector.tensor_sub(accs, accs, e_t)
                elif mode == "grp":
                    gi, slot = grp_of[t]
                    if slot != 3:
                        return     # one CCE DMA per completed group of 4
                    e4 = e4_tiles.pop(gi)
                    dst = acc_dp if tile_sign(t) > 0 else acc_dn
                    nc.gpsimd.dma_start(
                        out=dst.unsqueeze(1).broadcast_to([128, 4, IC]),
                        in_=e4,
                        accum_op=mybir.AluOpType.add,
                    )
                elif mode == "stt":
                    nc.vector.scalar_tensor_tensor(
                        out=accs, in0=e_t, scalar=sgn_sb[:, 0:1], in1=accs,
                        op0=mybir.AluOpType.mult, op1=mybir.AluOpType.add,
                    )
                else:
                    w = ones_sb if tile_sign(t) > 0 else mones_sb
                    for ic in range(2):
                        sl = slice(ic * 512, (ic + 1) * 512)
                        nc.tensor.matmul(
                            out=s_ps[:, sl], lhsT=w, rhs=e_t[:, sl],
                            start=first_mv[0], stop=False,
                            skip_group_check=True,
                        )
                    first_mv[0] = False

            for t in range(JT):
                g_ps = emit_gemm(t)
                emit_exp(t, g_ps)
                if t >= ACC_LAG:
                    emit_acc(t - ACC_LAG)
            for t in range(JT - ACC_LAG, JT):
                emit_acc(t)

            # Fold the DVE/DMA accumulators in: s[0,i] += sum_p acc[p,i];
            # the last matmul stops the PSUM accumulation group.
            folds = [(accs, ones_sb)]
            if groups:
                folds += [(acc_dp, ones_sb), (acc_dn, mones_sb)]
            for fi, (f, w) in enumerate(folds):
                for ic in range(2):
                    sl = slice(ic * 512, (ic + 1) * 512)
                    last = fi == len(folds) - 1 and ic == 1
                    nc.tensor.matmul(
                        out=s_ps[:, sl], lhsT=w, rhs=f[:, sl],
                        start=first_mv[0], stop=last,
                        skip_group_check=True,
                    )
                    first_mv[0] = False
            if DEBUG_S:
                s_sb = singles.tile([1, IC], F32, tag="s_sb")
                nc.vector.tensor_copy(s_sb, s_ps)
                nc.sync.dma_start(out=s_out[:, :], in_=s_sb)  # pre-mask s
            p_row = singles.tile([1, IC], F32, tag="p_row")
            nc.vector.tensor_mul(p_row, s_ps, e_row)
            if not b_is_zero:
                nc.vector.tensor_scalar(
                    out=p_row, in0=p_row, scalar1=b_sb[0:1, 0:1], scalar2=None,
                    op0=mybir.AluOpType.add,
                )
            nc.sync.dma_start(out=out[:, :], in_=p_row)

    nc.finalize()
    return nc


def kernel(X, X_train, alphas, y_train, b):
    X = np.ascontiguousarray(np.asarray(X, dtype=np.float32))
    X_train = np.ascontiguousarray(np.asarray(X_train, dtype=np.float32))
    alphas = np.asarray(alphas, dtype=np.float32).reshape(M)
    y_train = np.asarray(y_train, dtype=np.float32).reshape(M)
    b_arr = np.asarray(b, dtype=np.float32).reshape(1, 1)

    # Sort train points by label (+1 first), then by c within each label so
    # c values on a partition row are close (enables shared-bias tricks).
    c_all = (-GAMMA * (X_train * X_train).sum(1)
             + np.log(np.maximum(alphas, np.float32(1e-38)))).astype(np.float32)
    perm = np.lexsort((c_all, -y_train))
    n_pos = int((y_train > 0).sum())
    Xt_p = X_train[perm]
    c = c_all[perm]

    cj = np.ascontiguousarray(c.reshape(JT, 128).T)          # [128, JT]
    bcol = (SCH_A * cj + np.float32(SCH_B)).astype(np.float32)
    r = n_pos % 128
    sgn_vec = np.where(np.arange(128) < r, 1.0, -1.0).astype(
        ml_dtypes.bfloat16).reshape(128, 1)

    # fp8 DoubleRow layouts: [ki, ksub, col], d = ksub*128 + ki.
    f8 = ml_dtypes.float8_e4m3fn
    xt_dr = np.ascontiguousarray(
        Xt_p.T.reshape(2, 128, M).transpose(1, 0, 2).astype(f8))
    nxsq_full = (-GAMMA * (X * X).sum(1)).astype(np.float32)

    in_maps = []
    for k in range(NCORES):
        sl = slice(k * IC, (k + 1) * IC)
        x_dr = np.ascontiguousarray(
            X[sl].T.reshape(2, 128, IC).transpose(1, 0, 2).astype(f8))
        in_maps.append({
            "xt_t": xt_dr,
            "x_t": x_dr,
            "cj": cj,
            "bcol": bcol,
            "sgn": sgn_vec,
            "nxsq": np.ascontiguousarray(nxsq_full[sl].reshape(1, IC)),
            "bb": b_arr,
        })

    nc = _build_program(n_pos, b_is_zero=float(b_arr.reshape(-1)[0]) == 0.0)
    res = run_bass_kernel_spmd(nc, in_maps, list(range(NCORES)), trace=TRACE)
    global LAST_RESULTS
    LAST_RESULTS = res

    preds = np.concatenate([res.results[k]["out"][0] for k in range(NCORES)])
    return preds.reshape(N, 1).astype(np.float32)


# revision 45
# speedup vs baseline: 1.1785x; 1.0063x over previous
"""RBF-kernel SVM prediction on 8 Trainium2 NeuronCores.

predictions = exp(-g*||x_i - t_j||^2) @ (alphas*y) + b,  g = 0.5

Strategy (per sharding hint): shard X rows 8-way, replicate train side.
Math is factorized as
    pred_i = exp(-g*||x_i||^2) * sum_j y_j * exp(x_i . t_j + c_j) + b
    c_j    = -g*||t_j||^2 + ln(alpha_j)
so the train-side affine terms ride per-partition biases and the
query-side factor is a per-row epilogue scale. Train points are host-sorted
by label so the +/- y_j signs become whole-tile add/sub.

Per-core engine balance (the loop is exp-throughput bound):
 - PE runs the G = Xt.X^T GEMM in fp8 DoubleRow (2x rate), plus weighted
   ones-matvecs that accumulate ~half the exp'd tiles straight into a
   persistent PSUM row (one accumulation group across the whole loop).
 - ACT does true exp for ~2/3 of tiles (PSUM source, per-partition bias).
 - DVE runs a custom fused op for the rest, computing bf16(e^x) bit
   patterns directly (Schraudolph: bits16 = clamp(A*(G+c)+B, 0, 32000)
   as uint16; the fp32 clamp precedes conversion so no NaN/Inf patterns
   can appear), plus bf16 tensor-tensor accumulates. Exps of DVE-acc'd
   tiles land pairwise in [128, 2048] tiles so one TT covers two j-tiles.
GPSIMD is left idle: it shares an SBUF port with the DVE and measured
as dragging all DVE 2x ops down to 1x when used for accumulates.
"""

import os
import sys

import numpy as np

for _p in ("/opt/trn_rl_repo", "/root/.axon_site/_ro/trn_rl_repo"):
    if os.path.isdir(_p) and _p not in sys.path:
        sys.path.append(_p)

import ml_dtypes

import concourse.bass as bass
import concourse.tile as tile
from concourse import bacc, mybir
from concourse.bass_utils import run_bass_kernel_spmd

GAMMA = 0.5
N, M, D = 8192, 8192, 256
NCORES = 8
IC = N // NCORES          # query rows per core (1024)
JT = M // 128             # j-tiles (64)
F32 = mybir.dt.float32
BF16 = mybir.dt.bfloat16
FP8 = mybir.dt.float8e4
U16 = mybir.dt.uint16
FP32_MIN_NORMAL = 1.1754944e-38

# Schraudolph constants for bf16: bits16(e^x) ~ A*x + B, clamped to [0, CLAMP_HI].
SCH_A = 184.6650784   # 128 / ln(2)
SCH_B = 16250.496     # 128 * (127 - 0.0430)
SCH_CLAMP = 32000.0   # < 0x7F80 (inf); e^x here never exceeds e^10 anyway

def _env(name, default):
    return int(os.environ.get(name, default))


N_MAGIC = _env("KN_MAGIC", 22)    # tiles exp'd on the DVE
N_DMA_GRP = _env("KN_DMAGRP", 0)  # groups of 4 tiles accumulated by CCE DMAs
N_WARM = _env("KN_WARM", 10)      # PE warmup matmuls (HAM clock-gate release)
MV_TAIL = _env("KN_MVTAIL", 56)   # tiles >= this always accumulate via PE matvec
ACC_LAG = _env("KN_LAG", 2)       # emit tile t's acc after tile t+ACC_LAG's GEMM
EPOOL = _env("KN_EPOOL", 6)       # e-tile pool buffers

# Set by test harness to collect a profile; harness grading leaves it off.
TRACE = False
DEBUG_S = True            # emit the pre-mask s row for test-side validation
LAST_RESULTS = None

_EXP_OP = None


def _register_exp_op():
    """Register the custom DVE op computing bf16 exp bit patterns.

    body = min(max(Src0*C0 + C1, 0), C2), written to a uint16 tile whose
    bits, reinterpreted as bf16, approximate e^(Src0 + c) (C1 carries the
    per-partition c bias pre-scaled by A).
    """
    global _EXP_OP
    if _EXP_OP is not None:
        return _EXP_OP
    from concourse import dve_ops as dvo
    from concourse.dve_ops import DveOp
    from concourse.dve_spec import Spec, Src0, C0, C1, C2, Zero, maxx, minn, lower
    from concourse.dve_uop import DveOpSpec

    name = "EXP_BITS_U16_ANT"
    for op in dvo.OPS:
        if op.name == name:
            _EXP_OP = op
            return op
    spec = Spec(
        body=minn(maxx(Src0 * C0 + C1, Zero), C2),
        reference=lambda in0, in1, s0, s1, imm2: np.minimum(
            np.maximum(in0.astype(np.float32) * s0 + s1, 0.0), imm2
        ),
    )
    opcode = dvo._CUSTOM_DVE_ROW_BASE + len(dvo.OPS)
    shas = {}
    for ver in ("v3", "v4"):
        tmp = DveOpSpec(name=name, opcode=opcode, uops=lower(spec, ver=ver),
                        rd1_en=False)
        shas[ver] = tmp.sha(ver)
    op = DveOp(name, spec, subdim=False, uops_sha=shas)
    dvo.OPS.append(op)
    dvo.CUSTOM_DVE_SPECS[name] = spec
    dvo._SUB_OPCODE_FOR_NAME[name] = opcode
    _EXP_OP = op
    return op


def _plan_tiles(n_pos: int):
    """Per-tile engine assignment.

    Returns (magic, acc_mode) where acc_mode[t] is one of 'mv' (PE
    matvec), 'dve' (DVE TT into accs), 'stt' (mixed-sign tile).
    """
    tb = n_pos // 128 if n_pos % 128 else -1
    # Spread DVE-exp tiles over [4, JT): the first tiles stay on ACT so the
    # pipeline fill isn't gated on the DVE finishing its startup memsets.
    magic = set(4 + int(round(i * (JT - 4) / N_MAGIC)) for i in range(N_MAGIC))
    k = 4
    while len(magic) < N_MAGIC:          # dedupe fallback
        if k not in magic:
            magic.add(k)
        k += 1

    acc_mode = {}
    for t in range(JT):
        if t == tb:
            acc_mode[t] = "stt"
        elif t % 2 == 0 or t >= MV_TAIL:
            acc_mode[t] = "mv"
        else:
            acc_mode[t] = "dve"

    # Form groups of 4 same-sign 'dve' tiles for batched CCE-DMA accumulate.
    dve_tiles = [t for t in range(JT) if acc_mode[t] == "dve"]
    groups = []
    cur = []
    for t in dve_tiles:
        if len(groups) >= N_DMA_GRP:
            break
        sign = +1 if (t + 1) * 128 <= n_pos else -1
        if cur and sign != cur[0][1]:
            cur = []
        cur.append((t, sign))
        if len(cur) == 4:
            groups.append([x[0] for x in cur])
            for x in cur:
                acc_mode[x[0]] = "grp"
            cur = []
    return magic, acc_mode, groups


def _build_program(n_pos: int, b_is_zero: bool):
    exp_op = _register_exp_op()
    magic, acc_mode, groups = _plan_tiles(n_pos)
    grp_of = {}
    for gi, g in enumerate(groups):
        for slot, t in enumerate(g):
            grp_of[t] = (gi, slot)
    nc = bacc.Bacc()

    # fp8 DoubleRow operands: [ki, ksub, col] with d = ksub*128 + ki.
    xt_t = nc.dram_tensor("xt_t", [128, 2, M], FP8, kind="ExternalInput")
    x_t = nc.dram_tensor("x_t", [128, 2, IC], FP8, kind="ExternalInput")
    cj = nc.dram_tensor("cj", [128, JT], F32, kind="ExternalInput")
    bcol = nc.dram_tensor("bcol", [128, JT], F32, kind="ExternalInput")
    sgn = nc.dram_tensor("sgn", [128, 1], BF16, kind="ExternalInput")
    nxsq = nc.dram_tensor("nxsq", [1, IC], F32, kind="ExternalInput")
    bb = nc.dram_tensor("bb", [1, 1], F32, kind="ExternalInput")
    out = nc.dram_tensor("out", [1, IC], F32, kind="ExternalOutput")
    s_out = nc.dram_tensor("s_out", [1, IC], F32, kind="ExternalOutput")

    NCHUNK = 16           # xt column chunks so matmuls wait on small DMAs
    CW = M // NCHUNK      # 512 j-columns per chunk

    tb = n_pos // 128 if n_pos % 128 else -1   # mixed-sign boundary tile

    def tile_sign(t):
        return +1 if (t + 1) * 128 <= n_pos else -1

    with tile.TileContext(nc) as tc:
        with (
            tc.tile_pool(name="singles", bufs=1) as singles,
            tc.tile_pool(name="epool", bufs=EPOOL) as epool,
            tc.tile_pool(name="gpsum", bufs=3, space="PSUM") as gpsum,
            tc.tile_pool(name="spsum", bufs=1, space="PSUM") as spsum,
        ):
            # Resident inputs, all on the sync HWDGE queue (the scalar queue
            # executes on the ACT engine and steals exp throughput), small
            # first-needed operands ahead of the bulk xt chunks.
            x_sb = singles.tile([128, 2, IC], FP8, tag="x")
            nc.sync.dma_start(out=x_sb, in_=x_t[:, :, :])
            cj_sb = singles.tile([128, JT], F32, tag="cj")
            nc.sync.dma_start(out=cj_sb, in_=cj[:, :])
            bcol_sb = singles.tile([128, JT], F32, tag="bcol")
            nc.sync.dma_start(out=bcol_sb, in_=bcol[:, :])
            sgn_sb = singles.tile([128, 1], BF16, tag="sgn")
            nc.sync.dma_start(out=sgn_sb, in_=sgn[:, :])
            nxsq_sb = singles.tile([1, IC], F32, tag="nxsq")
            nc.sync.dma_start(out=nxsq_sb, in_=nxsq[:, :])
            b_sb = singles.tile([1, 1], F32, tag="b")
            nc.sync.dma_start(out=b_sb, in_=bb[:, :])
            xt_sb = [None] * NCHUNK
            for ck in range(NCHUNK):
                t = singles.tile([128, 2, CW], FP8, tag=f"xt_{ck}")
                nc.sync.dma_start(
                    out=t, in_=xt_t[:, :, ck * CW:(ck + 1) * CW],
                )
                xt_sb[ck] = t

            # Warm the PE while input DMAs are in flight (HAM clock gate).
            # warm_w memset is first so warmups aren't gated on other memsets.
            warm_w = singles.tile([128, 128], BF16, tag="warm_w")
            nc.vector.memset(warm_w, 0.0)
            warm_ps = spsum.tile([1, 128], F32, tag="s")
            for _ in range(N_WARM):
                nc.tensor.matmul(
                    out=warm_ps, lhsT=warm_w[:, 0:1], rhs=warm_w[:, :],
                    start=True, stop=True,
                )

            ones_sb = singles.tile([128, 1], BF16, tag="ones")
            nc.vector.memset(ones_sb, 1.0)
            mones_sb = singles.tile([128, 1], BF16, tag="mones")
            nc.vector.memset(mones_sb, -1.0)
            accs = singles.tile([128, IC], BF16, tag="accs")
            nc.vector.memset(accs, 0.0)
            if groups:
                acc_dp = singles.tile([128, IC], BF16, tag="acc_dp")
                nc.vector.memset(acc_dp, 0.0)
                acc_dn = singles.tile([128, IC], BF16, tag="acc_dn")
                nc.vector.memset(acc_dn, 0.0)

            # Query-side factor, computed early so ACT's table load happens
            # during the DMA window.
            e_row = singles.tile([1, IC], F32, tag="e_row")
            nc.scalar.activation(
                out=e_row, in_=nxsq_sb, func=mybir.ActivationFunctionType.Exp
            )
            # Emulate fp32 FTZ on the factor: the reference's direct
            # exp(-g*d) underflows to 0; keep the factored path bit-identical.
            m_row = singles.tile([1, IC], F32, tag="m_row")
            nc.vector.tensor_scalar(
                out=m_row, in0=e_row, scalar1=FP32_MIN_NORMAL, scalar2=None,
                op0=mybir.AluOpType.is_ge,
            )
            nc.vector.tensor_mul(e_row, e_row, m_row)

            # s_ps accumulates PE-matvec'd tiles across the whole j-loop and
            # receives the folds of acc2/accs at the end. Shares the "s"
            # PSUM slot with warm_ps; the first matvec's start=True clears it.
            s_ps = spsum.tile([1, IC], F32, tag="s")

            e_views = {}          # t -> bf16-view AP of tile t's exp output
            e4_tiles = {}         # group idx -> wide [128, 4, IC] tile
            first_mv = [True]

            def emit_gemm(t):
                ck, col = t // 4, (t % 4) * 128
                g_ps = gpsum.tile([128, IC], F32, tag="g", name=f"g{t}")
                for ic in range(2):
                    sl = slice(ic * 512, (ic + 1) * 512)
                    nc.tensor.matmul(
                        out=g_ps[:, sl],
                        lhsT=xt_sb[ck][:, :, col:col + 128],
                        rhs=x_sb[:, :, sl],
                        start=True, stop=True,
                        perf_mode=mybir.MatmulPerfMode.DoubleRow,
                    )
                return g_ps

            def emit_exp(t, g_ps):
                if t in grp_of:
                    gi, slot = grp_of[t]
                    if gi not in e4_tiles:
                        e4_tiles[gi] = epool.tile(
                            [128, 4, IC], BF16, tag="e4", bufs=2, name=f"e4_{gi}")
                    dest = e4_tiles[gi][:, slot, :]
                else:
                    dest = epool.tile([128, IC], BF16, tag="e", name=f"e{t}")
                if t in magic:
                    nc.vector._custom_dve(
                        exp_op, out=dest.bitcast(U16), in0=g_ps,
                        s0=SCH_A, s1=bcol_sb[:, t:t + 1], imm2=SCH_CLAMP,
                    )
                else:
                    nc.scalar.activation(
                        out=dest, in_=g_ps,
                        func=mybir.ActivationFunctionType.Exp,
                        bias=cj_sb[:, t:t + 1], scale=1.0,
                    )
                e_views[t] = dest

            def emit_acc(t):
                mode = acc_mode[t]
                e_t = e_views.pop(t)
                if mode == "dve":
                    if tile_sign(t) > 0:
                        nc.vector.tensor_add(accs, accs, e_t)
                    else:
                        nc.vector.tensor_sub(accs, accs, e_t)
                elif mode == "grp":
                    gi, slot = grp_of[t]
                    if slot != 3:
                        return     # one CCE DMA per completed group of 4
                    e4 = e4_tiles.pop(gi)
                    dst = acc_dp if tile_sign(t) > 0 else acc_dn
                    nc.gpsimd.dma_start(
                        out=dst.unsqueeze(1).broadcast_to([128, 4, IC]),
                        in_=e4,
                        accum_op=mybir.AluOpType.add,
                    )
                elif mode == "stt":
                    nc.vector.scalar_tensor_tensor(
                        out=accs, in0=e_t, scalar=sgn_sb[:, 0:1], in1=accs,
                        op0=mybir.AluOpType.mult, op1=mybir.AluOpType.add,
                    )
                else:
                    w = ones_sb if tile_sign(t) > 0 else mones_sb
                    for ic in range(2):
                        sl = slice(ic * 512, (ic + 1) * 512)
                        nc.tensor.matmul(
                            out=s_ps[:, sl], lhsT=w, rhs=e_t[:, sl],
                            start=first_mv[0], stop=False,
                            skip_group_check=True,
                        )
                    first_mv[0] = False

            for t in range(JT):
                g_ps = emit_gemm(t)
                emit_exp(t, g_ps)
                if t >= ACC_LAG:
                    emit_acc(t - ACC_LAG)
            for t in range(JT - ACC_LAG, JT):
                emit_acc(t)

            # Fold the DVE/DMA accumulators in: s[0,i] += sum_p acc[p,i];
            # the last matmul stops the PSUM accumulation group.
            folds = [(accs, ones_sb)]
            if groups:
                folds += [(acc_dp, ones_sb), (acc_dn, mones_sb)]
            for fi, (f, w) in enumerate(folds):
                for ic in range(2):
                    sl = slice(ic * 512, (ic + 1) * 512)
                    last = fi == len(folds) - 1 and ic == 1
                    nc.tensor.matmul(
                        out=s_ps[:, sl], lhsT=w, rhs=f[:, sl],
                        start=first_mv[0], stop=last,
                        skip_group_check=True,
                    )
                    first_mv[0] = False
            if DEBUG_S:
                s_sb = singles.tile([1, IC], F32, tag="s_sb")
                nc.vector.tensor_copy(s_sb, s_ps)
                nc.sync.dma_start(out=s_out[:, :], in_=s_sb)  # pre-mask s
            p_row = singles.tile([1, IC], F32, tag="p_row")
            nc.vector.tensor_mul(p_row, s_ps, e_row)
            if not b_is_zero:
                nc.vector.tensor_scalar(
                    out=p_row, in0=p_row, scalar1=b_sb[0:1, 0:1], scalar2=None,
                    op0=mybir.AluOpType.add,
                )
            nc.sync.dma_start(out=out[:, :], in_=p_row)

    nc.finalize()
    return nc


def kernel(X, X_train, alphas, y_train, b):
    X = np.ascontiguousarray(np.asarray(X, dtype=np.float32))
    X_train = np.ascontiguousarray(np.asarray(X_train, dtype=np.float32))
    alphas = np.asarray(alphas, dtype=np.float32).reshape(M)
    y_train = np.asarray(y_train, dtype=np.float32).reshape(M)
    b_arr = np.asarray(b, dtype=np.float32).reshape(1, 1)

    # Sort train points by label (+1 first), then by c within each label so
    # c values on a partition row are close (enables shared-bias tricks).
    c_all = (-GAMMA * (X_train * X_train).sum(1)
             + np.log(np.maximum(alphas, np.float32(1e-38)))).astype(np.float32)
    perm = np.lexsort((c_all, -y_train))
    n_pos = int((y_train > 0).sum())
    Xt_p = X_train[perm]
    c = c_all[perm]

    cj = np.ascontiguousarray(c.reshape(JT, 128).T)          # [128, JT]
    bcol = (SCH_A * cj + np.float32(SCH_B)).astype(np.float32)
    r = n_pos % 128
    sgn_vec = np.where(np.arange(128) < r, 1.0, -1.0).astype(
        ml_dtypes.bfloat16).reshape(128, 1)

    # fp8 DoubleRow layouts: [ki, ksub, col], d = ksub*128 + ki.
    f8 = ml_dtypes.float8_e4m3fn
    xt_dr = np.ascontiguousarray(
        Xt_p.T.reshape(2, 128, M).transpose(1, 0, 2).astype(f8))
    nxsq_full = (-GAMMA * (X * X).sum(1)).astype(np.float32)

    in_maps = []
    for k in range(NCORES):
        sl = slice(k * IC, (k + 1) * IC)
        x_dr = np.ascontiguousarray(
            X[sl].T.reshape(2, 128, IC).transpose(1, 0, 2).astype(f8))
        in_maps.append({
            "xt_t": xt_dr,
            "x_t": x_dr,
            "cj": cj,
            "bcol": bcol,
            "sgn": sgn_vec,
            "nxsq": np.ascontiguousarray(nxsq_full[sl].reshape(1, IC)),
            "bb": b_arr,
        })

    nc = _build_program(n_pos, b_is_zero=float(b_arr.reshape(-1)[0]) == 0.0)
    res = run_bass_kernel_spmd(nc, in_maps, list(range(NCORES)), trace=TRACE)
    global LAST_RESULTS
    LAST_RESULTS = res

    preds = np.concatenate([res.results[k]["out"][0] for k in range(NCORES)])
    return preds.reshape(N, 1).astype(np.float32)


# revision 47
# speedup vs baseline: 1.2069x; 1.0241x over previous
"""RBF-kernel SVM prediction on 8 Trainium2 NeuronCores.

predictions = exp(-g*||x_i - t_j||^2) @ (alphas*y) + b,  g = 0.5

Strategy (per sharding hint): shard X rows 8-way, replicate train side.
Math is factorized as
    pred_i = exp(-g*||x_i||^2) * sum_j y_j * exp(x_i . t_j + c_j) + b
    c_j    = -g*||t_j||^2 + ln(alpha_j)
so the train-side affine terms ride per-partition biases and the
query-side factor is a per-row epilogue scale. Train points are host-sorted
by label so the +/- y_j signs become whole-tile add/sub.

Per-core engine balance (the loop is exp-throughput bound):
 - PE runs the G = Xt.X^T GEMM in fp8 DoubleRow (2x rate), plus weighted
   ones-matvecs that accumulate ~half the exp'd tiles straight into a
   persistent PSUM row (one accumulation group across the whole loop).
 - ACT does true exp for ~2/3 of tiles (PSUM source, per-partition bias).
 - DVE runs a custom fused op for the rest, computing bf16(e^x) bit
   patterns directly (Schraudolph: bits16 = clamp(A*(G+c)+B, 0, 32000)
   as uint16; the fp32 clamp precedes conversion so no NaN/Inf patterns
   can appear), plus bf16 tensor-tensor accumulates. Exps of DVE-acc'd
   tiles land pairwise in [128, 2048] tiles so one TT covers two j-tiles.
GPSIMD is left idle: it shares an SBUF port with the DVE and measured
as dragging all DVE 2x ops down to 1x when used for accumulates.
"""

import os
import sys

import numpy as np

for _p in ("/opt/trn_rl_repo", "/root/.axon_site/_ro/trn_rl_repo"):
    if os.path.isdir(_p) and _p not in sys.path:
        sys.path.append(_p)

import ml_dtypes

import concourse.bass as bass
import concourse.tile as tile
from concourse import bacc, mybir
from concourse.bass_utils import run_bass_kernel_spmd

GAMMA = 0.5
N, M, D = 8192, 8192, 256
NCORES = 8
IC = N // NCORES          # query rows per core (1024)
JT = M // 128             # j-tiles (64)
F32 = mybir.dt.float32
BF16 = mybir.dt.bfloat16
FP8 = mybir.dt.float8e4
U16 = mybir.dt.uint16
FP32_MIN_NORMAL = 1.1754944e-38

# Schraudolph constants for bf16: bits16(e^x) ~ A*x + B, clamped to [0, CLAMP_HI].
SCH_A = 184.6650784   # 128 / ln(2)
SCH_B = 16250.496     # 128 * (127 - 0.0430)
SCH_CLAMP = 32000.0   # < 0x7F80 (inf); e^x here never exceeds e^10 anyway

def _env(name, default):
    return int(os.environ.get(name, default))


N_MAGIC = _env("KN_MAGIC", 21)    # tiles exp'd on the DVE
N_DMA_GRP = _env("KN_DMAGRP", 0)  # groups of 4 tiles accumulated by CCE DMAs
N_WARM = _env("KN_WARM", 10)      # PE warmup matmuls (HAM clock-gate release)
MV_TAIL = _env("KN_MVTAIL", 56)   # tiles >= this always accumulate via PE matvec
ACC_LAG = _env("KN_LAG", 2)       # emit tile t's acc after tile t+ACC_LAG's GEMM
EPOOL = _env("KN_EPOOL", 6)       # e-tile pool buffers

# Set by test harness to collect a profile; harness grading leaves it off.
TRACE = False
DEBUG_S = os.environ.get("KN_DEBUG_S", "") != ""  # emit pre-mask s for tests
LAST_RESULTS = None

_EXP_OP = None


def _register_exp_op():
    """Register the custom DVE op computing bf16 exp bit patterns.

    body = min(max(Src0*C0 + C1, 0), C2), written to a uint16 tile whose
    bits, reinterpreted as bf16, approximate e^(Src0 + c) (C1 carries the
    per-partition c bias pre-scaled by A).
    """
    global _EXP_OP
    if _EXP_OP is not None:
        return _EXP_OP
    from concourse import dve_ops as dvo
    from concourse.dve_ops import DveOp
    from concourse.dve_spec import Spec, Src0, C0, C1, C2, Zero, maxx, minn, lower
    from concourse.dve_uop import DveOpSpec

    name = "EXP_BITS_U16_ANT"
    for op in dvo.OPS:
        if op.name == name:
            _EXP_OP = op
            return op
    spec = Spec(
        body=minn(maxx(Src0 * C0 + C1, Zero), C2),
        reference=lambda in0, in1, s0, s1, imm2: np.minimum(
            np.maximum(in0.astype(np.float32) * s0 + s1, 0.0), imm2
        ),
    )
    opcode = dvo._CUSTOM_DVE_ROW_BASE + len(dvo.OPS)
    shas = {}
    for ver in ("v3", "v4"):
        tmp = DveOpSpec(name=name, opcode=opcode, uops=lower(spec, ver=ver),
                        rd1_en=False)
        shas[ver] = tmp.sha(ver)
    op = DveOp(name, spec, subdim=False, uops_sha=shas)
    dvo.OPS.append(op)
    dvo.CUSTOM_DVE_SPECS[name] = spec
    dvo._SUB_OPCODE_FOR_NAME[name] = opcode
    _EXP_OP = op
    return op


def _plan_tiles(n_pos: int):
    """Per-tile engine assignment.

    Returns (magic, acc_mode) where acc_mode[t] is one of 'mv' (PE
    matvec), 'dve' (DVE TT into accs), 'stt' (mixed-sign tile).
    """
    tb = n_pos // 128 if n_pos % 128 else -1
    # Spread DVE-exp tiles over [4, JT): the first tiles stay on ACT so the
    # pipeline fill isn't gated on the DVE finishing its startup memsets.
    magic = set(4 + int(round(i * (JT - 4) / N_MAGIC)) for i in range(N_MAGIC))
    k = 4
    while len(magic) < N_MAGIC:          # dedupe fallback
        if k not in magic:
            magic.add(k)
        k += 1

    acc_mode = {}
    for t in range(JT):
        if t == tb:
            acc_mode[t] = "stt"
        elif t % 2 == 0 or t >= MV_TAIL:
            acc_mode[t] = "mv"
        else:
            acc_mode[t] = "dve"

    # Form groups of 4 same-sign 'dve' tiles for batched CCE-DMA accumulate.
    dve_tiles = [t for t in range(JT) if acc_mode[t] == "dve"]
    groups = []
    cur = []
    for t in dve_tiles:
        if len(groups) >= N_DMA_GRP:
            break
        sign = +1 if (t + 1) * 128 <= n_pos else -1
        if cur and sign != cur[0][1]:
            cur = []
        cur.append((t, sign))
        if len(cur) == 4:
            groups.append([x[0] for x in cur])
            for x in cur:
                acc_mode[x[0]] = "grp"
            cur = []
    return magic, acc_mode, groups


def _build_program(n_pos: int, b_is_zero: bool):
    exp_op = _register_exp_op()
    magic, acc_mode, groups = _plan_tiles(n_pos)
    grp_of = {}
    for gi, g in enumerate(groups):
        for slot, t in enumerate(g):
            grp_of[t] = (gi, slot)
    nc = bacc.Bacc()

    # fp8 DoubleRow operands: [ki, ksub, col] with d = ksub*128 + ki.
    xt_t = nc.dram_tensor("xt_t", [128, 2, M], FP8, kind="ExternalInput")
    x_t = nc.dram_tensor("x_t", [128, 2, IC], FP8, kind="ExternalInput")
    cj = nc.dram_tensor("cj", [128, JT], F32, kind="ExternalInput")
    bcol = nc.dram_tensor("bcol", [128, JT], F32, kind="ExternalInput")
    sgn = nc.dram_tensor("sgn", [128, 1], BF16, kind="ExternalInput")
    nxsq = nc.dram_tensor("nxsq", [1, IC], F32, kind="ExternalInput")
    bb = nc.dram_tensor("bb", [1, 1], F32, kind="ExternalInput")
    out = nc.dram_tensor("out", [1, IC], F32, kind="ExternalOutput")
    s_out = nc.dram_tensor("s_out", [1, IC], F32, kind="ExternalOutput")

    NCHUNK = 16           # xt column chunks so matmuls wait on small DMAs
    CW = M // NCHUNK      # 512 j-columns per chunk

    tb = n_pos // 128 if n_pos % 128 else -1   # mixed-sign boundary tile

    def tile_sign(t):
        return +1 if (t + 1) * 128 <= n_pos else -1

    with tile.TileContext(nc) as tc:
        with (
            tc.tile_pool(name="singles", bufs=1) as singles,
            tc.tile_pool(name="epool", bufs=EPOOL) as epool,
            tc.tile_pool(name="gpsum", bufs=3, space="PSUM") as gpsum,
            tc.tile_pool(name="spsum", bufs=1, space="PSUM") as spsum,
        ):
            # Resident inputs, all on the sync HWDGE queue (the scalar queue
            # executes on the ACT engine and steals exp throughput), small
            # first-needed operands ahead of the bulk xt chunks.
            x_sb = singles.tile([128, 2, IC], FP8, tag="x")
            nc.sync.dma_start(out=x_sb, in_=x_t[:, :, :])
            cj_sb = singles.tile([128, JT], F32, tag="cj")
            nc.sync.dma_start(out=cj_sb, in_=cj[:, :])
            bcol_sb = singles.tile([128, JT], F32, tag="bcol")
            nc.sync.dma_start(out=bcol_sb, in_=bcol[:, :])
            sgn_sb = singles.tile([128, 1], BF16, tag="sgn")
            nc.sync.dma_start(out=sgn_sb, in_=sgn[:, :])
            nxsq_sb = singles.tile([1, IC], F32, tag="nxsq")
            nc.sync.dma_start(out=nxsq_sb, in_=nxsq[:, :])
            b_sb = singles.tile([1, 1], F32, tag="b")
            nc.sync.dma_start(out=b_sb, in_=bb[:, :])
            xt_sb = [None] * NCHUNK
            for ck in range(NCHUNK):
                t = singles.tile([128, 2, CW], FP8, tag=f"xt_{ck}")
                nc.sync.dma_start(
                    out=t, in_=xt_t[:, :, ck * CW:(ck + 1) * CW],
                )
                xt_sb[ck] = t

            # Warm the PE while input DMAs are in flight (HAM clock gate).
            # warm_w memset is first so warmups aren't gated on other memsets.
            warm_w = singles.tile([128, 128], BF16, tag="warm_w")
            nc.vector.memset(warm_w, 0.0)
            warm_ps = spsum.tile([1, 128], F32, tag="s")
            for _ in range(N_WARM):
                nc.tensor.matmul(
                    out=warm_ps, lhsT=warm_w[:, 0:1], rhs=warm_w[:, :],
                    start=True, stop=True,
                )

            ones_sb = singles.tile([128, 1], BF16, tag="ones")
            nc.vector.memset(ones_sb, 1.0)
            mones_sb = singles.tile([128, 1], BF16, tag="mones")
            nc.vector.memset(mones_sb, -1.0)
            accs = singles.tile([128, IC], BF16, tag="accs")
            nc.vector.memset(accs, 0.0)
            if groups:
                acc_dp = singles.tile([128, IC], BF16, tag="acc_dp")
                nc.vector.memset(acc_dp, 0.0)
                acc_dn = singles.tile([128, IC], BF16, tag="acc_dn")
                nc.vector.memset(acc_dn, 0.0)

            # Query-side factor, computed early so ACT's table load happens
            # during the DMA window.
            e_row = singles.tile([1, IC], F32, tag="e_row")
            nc.scalar.activation(
                out=e_row, in_=nxsq_sb, func=mybir.ActivationFunctionType.Exp
            )
            # Emulate fp32 FTZ on the factor: the reference's direct
            # exp(-g*d) underflows to 0; keep the factored path bit-identical.
            m_row = singles.tile([1, IC], F32, tag="m_row")
            nc.vector.tensor_scalar(
                out=m_row, in0=e_row, scalar1=FP32_MIN_NORMAL, scalar2=None,
                op0=mybir.AluOpType.is_ge,
            )
            nc.vector.tensor_mul(e_row, e_row, m_row)

            # s_ps accumulates PE-matvec'd tiles across the whole j-loop and
            # receives the folds of acc2/accs at the end. Shares the "s"
            # PSUM slot with warm_ps; the first matvec's start=True clears it.
            s_ps = spsum.tile([1, IC], F32, tag="s")

            e_views = {}          # t -> bf16-view AP of tile t's exp output
            e4_tiles = {}         # group idx -> wide [128, 4, IC] tile
            first_mv = [True]

            def emit_gemm(t):
                ck, col = t // 4, (t % 4) * 128
                g_ps = gpsum.tile([128, IC], F32, tag="g", name=f"g{t}")
                for ic in range(2):
                    sl = slice(ic * 512, (ic + 1) * 512)
                    nc.tensor.matmul(
                        out=g_ps[:, sl],
                        lhsT=xt_sb[ck][:, :, col:col + 128],
                        rhs=x_sb[:, :, sl],
                        start=True, stop=True,
                        perf_mode=mybir.MatmulPerfMode.DoubleRow,
                    )
                return g_ps

            def emit_exp(t, g_ps):
                if t in grp_of:
                    gi, slot = grp_of[t]
                    if gi not in e4_tiles:
                        e4_tiles[gi] = epool.tile(
                            [128, 4, IC], BF16, tag="e4", bufs=2, name=f"e4_{gi}")
                    dest = e4_tiles[gi][:, slot, :]
                else:
                    dest = epool.tile([128, IC], BF16, tag="e", name=f"e{t}")
                if t in magic:
                    nc.vector._custom_dve(
                        exp_op, out=dest.bitcast(U16), in0=g_ps,
                        s0=SCH_A, s1=bcol_sb[:, t:t + 1], imm2=SCH_CLAMP,
                    )
                else:
                    nc.scalar.activation(
                        out=dest, in_=g_ps,
                        func=mybir.ActivationFunctionType.Exp,
                        bias=cj_sb[:, t:t + 1], scale=1.0,
                    )
                e_views[t] = dest

            def emit_acc(t):
                mode = acc_mode[t]
                e_t = e_views.pop(t)
                if mode == "dve":
                    if tile_sign(t) > 0:
                        nc.vector.tensor_add(accs, accs, e_t)
                    else:
                        nc.vector.tensor_sub(accs, accs, e_t)
                elif mode == "grp":
                    gi, slot = grp_of[t]
                    if slot != 3:
                        return     # one CCE DMA per completed group of 4
                    e4 = e4_tiles.pop(gi)
                    dst = acc_dp if tile_sign(t) > 0 else acc_dn
                    nc.gpsimd.dma_start(
                        out=dst.unsqueeze(1).broadcast_to([128, 4, IC]),
                        in_=e4,
                        accum_op=mybir.AluOpType.add,
                    )
                elif mode == "stt":
                    nc.vector.scalar_tensor_tensor(
                        out=accs, in0=e_t, scalar=sgn_sb[:, 0:1], in1=accs,
                        op0=mybir.AluOpType.mult, op1=mybir.AluOpType.add,
                    )
                else:
                    w = ones_sb if tile_sign(t) > 0 else mones_sb
                    for ic in range(2):
                        sl = slice(ic * 512, (ic + 1) * 512)
                        nc.tensor.matmul(
                            out=s_ps[:, sl], lhsT=w, rhs=e_t[:, sl],
                            start=first_mv[0], stop=False,
                            skip_group_check=True,
                        )
                    first_mv[0] = False

            for t in range(JT):
                g_ps = emit_gemm(t)
                emit_exp(t, g_ps)
                if t >= ACC_LAG:
                    emit_acc(t - ACC_LAG)
            for t in range(JT - ACC_LAG, JT):
                emit_acc(t)

            # Fold the DVE/DMA accumulators in: s[0,i] += sum_p acc[p,i];
            # the last matmul stops the PSUM accumulation group.
            folds = [(accs, ones_sb)]
            if groups:
                folds += [(acc_dp, ones_sb), (acc_dn, mones_sb)]
            for fi, (f, w) in enumerate(folds):
                for ic in range(2):
                    sl = slice(ic * 512, (ic + 1) * 512)
                    last = fi == len(folds) - 1 and ic == 1
                    nc.tensor.matmul(
                        out=s_ps[:, sl], lhsT=w, rhs=f[:, sl],
                        start=first_mv[0], stop=last,
                        skip_group_check=True,
                    )
                    first_mv[0] = False
            if DEBUG_S:
                s_sb = singles.tile([1, IC], F32, tag="s_sb")
                nc.vector.tensor_copy(s_sb, s_ps)
                nc.sync.dma_start(out=s_out[:, :], in_=s_sb)  # pre-mask s
            p_row = singles.tile([1, IC], F32, tag="p_row")
            nc.vector.tensor_mul(p_row, s_ps, e_row)
            if not b_is_zero:
                nc.vector.tensor_scalar(
                    out=p_row, in0=p_row, scalar1=b_sb[0:1, 0:1], scalar2=None,
                    op0=mybir.AluOpType.add,
                )
            nc.sync.dma_start(out=out[:, :], in_=p_row)

    nc.finalize()
    return nc


def kernel(X, X_train, alphas, y_train, b):
    X = np.ascontiguousarray(np.asarray(X, dtype=np.float32))
    X_train = np.ascontiguousarray(np.asarray(X_train, dtype=np.float32))
    alphas = np.asarray(alphas, dtype=np.float32).reshape(M)
    y_train = np.asarray(y_train, dtype=np.float32).reshape(M)
    b_arr = np.asarray(b, dtype=np.float32).reshape(1, 1)

    # Sort train points by label (+1 first), then by c within each label so
    # c values on a partition row are close (enables shared-bias tricks).
    c_all = (-GAMMA * (X_train * X_train).sum(1)
             + np.log(np.maximum(alphas, np.float32(1e-38)))).astype(np.float32)
    perm = np.lexsort((c_all, -y_train))
    n_pos = int((y_train > 0).sum())
    Xt_p = X_train[perm]
    c = c_all[perm]

    cj = np.ascontiguousarray(c.reshape(JT, 128).T)          # [128, JT]
    bcol = (SCH_A * cj + np.float32(SCH_B)).astype(np.float32)
    r = n_pos % 128
    sgn_vec = np.where(np.arange(128) < r, 1.0, -1.0).astype(
        ml_dtypes.bfloat16).reshape(128, 1)

    # fp8 DoubleRow layouts: [ki, ksub, col], d = ksub*128 + ki.
    f8 = ml_dtypes.float8_e4m3fn
    xt_dr = np.ascontiguousarray(
        Xt_p.T.reshape(2, 128, M).transpose(1, 0, 2).astype(f8))
    nxsq_full = (-GAMMA * (X * X).sum(1)).astype(np.float32)

    in_maps = []
    for k in range(NCORES):
        sl = slice(k * IC, (k + 1) * IC)
        x_dr = np.ascontiguousarray(
            X[sl].T.reshape(2, 128, IC).transpose(1, 0, 2).astype(f8))
        in_maps.append({
            "xt_t": xt_dr,
            "x_t": x_dr,
            "cj": cj,
            "bcol": bcol,
            "sgn": sgn_vec,
            "nxsq": np.ascontiguousarray(nxsq_full[sl].reshape(1, IC)),
            "bb": b_arr,
        })

    nc = _build_program(n_pos, b_is_zero=float(b_arr.reshape(-1)[0]) == 0.0)
    res = run_bass_kernel_spmd(nc, in_maps, list(range(NCORES)), trace=TRACE)
    global LAST_RESULTS
    LAST_RESULTS = res

    preds = np.concatenate([res.results[k]["out"][0] for k in range(NCORES)])
    return preds.reshape(N, 1).astype(np.float32)


# revision 48
# speedup vs baseline: 1.2156x; 1.0072x over previous
"""RBF-kernel SVM prediction on 8 Trainium2 NeuronCores.

predictions = exp(-g*||x_i - t_j||^2) @ (alphas*y) + b,  g = 0.5

Strategy (per sharding hint): shard X rows 8-way, replicate train side.
Math is factorized as
    pred_i = exp(-g*||x_i||^2) * sum_j y_j * exp(x_i . t_j + c_j) + b
    c_j    = -g*||t_j||^2 + ln(alpha_j)
so the train-side affine terms ride per-partition biases and the
query-side factor is a per-row epilogue scale. Train points are host-sorted
by label so the +/- y_j signs become whole-tile add/sub.

Per-core engine balance (the loop is exp-throughput bound):
 - PE runs the G = Xt.X^T GEMM in fp8 DoubleRow (2x rate), plus weighted
   ones-matvecs that accumulate ~half the exp'd tiles straight into a
   persistent PSUM row (one accumulation group across the whole loop).
 - ACT does true exp for ~2/3 of tiles (PSUM source, per-partition bias).
 - DVE runs a custom fused op for the rest, computing bf16(e^x) bit
   patterns directly (Schraudolph: bits16 = clamp(A*(G+c)+B, 0, 32000)
   as uint16; the fp32 clamp precedes conversion so no NaN/Inf patterns
   can appear), plus bf16 tensor-tensor accumulates. Exps of DVE-acc'd
   tiles land pairwise in [128, 2048] tiles so one TT covers two j-tiles.
GPSIMD is left idle: it shares an SBUF port with the DVE and measured
as dragging all DVE 2x ops down to 1x when used for accumulates.
"""

import os
import sys

import numpy as np

for _p in ("/opt/trn_rl_repo", "/root/.axon_site/_ro/trn_rl_repo"):
    if os.path.isdir(_p) and _p not in sys.path:
        sys.path.append(_p)

import ml_dtypes

import concourse.bass as bass
import concourse.tile as tile
from concourse import bacc, mybir
from concourse.bass_utils import run_bass_kernel_spmd

GAMMA = 0.5
N, M, D = 8192, 8192, 256
NCORES = 8
IC = N // NCORES          # query rows per core (1024)
JT = M // 128             # j-tiles (64)
F32 = mybir.dt.float32
BF16 = mybir.dt.bfloat16
FP8 = mybir.dt.float8e4
U16 = mybir.dt.uint16
FP32_MIN_NORMAL = 1.1754944e-38

# Schraudolph constants for bf16: bits16(e^x) ~ A*x + B, clamped to [0, CLAMP_HI].
SCH_A = 184.6650784   # 128 / ln(2)
SCH_B = 16250.496     # 128 * (127 - 0.0430)
SCH_CLAMP = 32000.0   # < 0x7F80 (inf); e^x here never exceeds e^10 anyway

def _env(name, default):
    return int(os.environ.get(name, default))


N_MAGIC = _env("KN_MAGIC", 21)    # tiles exp'd on the DVE
N_DMA_GRP = _env("KN_DMAGRP", 0)  # groups of 4 tiles accumulated by CCE DMAs
N_WARM = _env("KN_WARM", 10)      # PE warmup matmuls (HAM clock-gate release)
MV_TAIL = _env("KN_MVTAIL", 60)   # tiles >= this always accumulate via PE matvec
ACC_LAG = _env("KN_LAG", 2)       # emit tile t's acc after tile t+ACC_LAG's GEMM
EPOOL = _env("KN_EPOOL", 6)       # e-tile pool buffers

# Set by test harness to collect a profile; harness grading leaves it off.
TRACE = False
DEBUG_S = os.environ.get("KN_DEBUG_S", "") != ""  # emit pre-mask s for tests
LAST_RESULTS = None

_EXP_OP = None


def _register_exp_op():
    """Register the custom DVE op computing bf16 exp bit patterns.

    body = min(max(Src0*C0 + C1, 0), C2), written to a uint16 tile whose
    bits, reinterpreted as bf16, approximate e^(Src0 + c) (C1 carries the
    per-partition c bias pre-scaled by A).
    """
    global _EXP_OP
    if _EXP_OP is not None:
        return _EXP_OP
    from concourse import dve_ops as dvo
    from concourse.dve_ops import DveOp
    from concourse.dve_spec import Spec, Src0, C0, C1, C2, Zero, maxx, minn, lower
    from concourse.dve_uop import DveOpSpec

    name = "EXP_BITS_U16_ANT"
    for op in dvo.OPS:
        if op.name == name:
            _EXP_OP = op
            return op
    spec = Spec(
        body=minn(maxx(Src0 * C0 + C1, Zero), C2),
        reference=lambda in0, in1, s0, s1, imm2: np.minimum(
            np.maximum(in0.astype(np.float32) * s0 + s1, 0.0), imm2
        ),
    )
    opcode = dvo._CUSTOM_DVE_ROW_BASE + len(dvo.OPS)
    shas = {}
    for ver in ("v3", "v4"):
        tmp = DveOpSpec(name=name, opcode=opcode, uops=lower(spec, ver=ver),
                        rd1_en=False)
        shas[ver] = tmp.sha(ver)
    op = DveOp(name, spec, subdim=False, uops_sha=shas)
    dvo.OPS.append(op)
    dvo.CUSTOM_DVE_SPECS[name] = spec
    dvo._SUB_OPCODE_FOR_NAME[name] = opcode
    _EXP_OP = op
    return op


def _plan_tiles(n_pos: int):
    """Per-tile engine assignment.

    Returns (magic, acc_mode) where acc_mode[t] is one of 'mv' (PE
    matvec), 'dve' (DVE TT into accs), 'stt' (mixed-sign tile).
    """
    tb = n_pos // 128 if n_pos % 128 else -1
    # Spread DVE-exp tiles over [4, JT): the first tiles stay on ACT so the
    # pipeline fill isn't gated on the DVE finishing its startup memsets.
    magic = set(4 + int(round(i * (JT - 4) / N_MAGIC)) for i in range(N_MAGIC))
    k = 4
    while len(magic) < N_MAGIC:          # dedupe fallback
        if k not in magic:
            magic.add(k)
        k += 1

    acc_mode = {}
    for t in range(JT):
        if t == tb:
            acc_mode[t] = "stt"
        elif t % 2 == 0 or t >= MV_TAIL:
            acc_mode[t] = "mv"
        else:
            acc_mode[t] = "dve"

    # Form groups of 4 same-sign 'dve' tiles for batched CCE-DMA accumulate.
    dve_tiles = [t for t in range(JT) if acc_mode[t] == "dve"]
    groups = []
    cur = []
    for t in dve_tiles:
        if len(groups) >= N_DMA_GRP:
            break
        sign = +1 if (t + 1) * 128 <= n_pos else -1
        if cur and sign != cur[0][1]:
            cur = []
        cur.append((t, sign))
        if len(cur) == 4:
            groups.append([x[0] for x in cur])
            for x in cur:
                acc_mode[x[0]] = "grp"
            cur = []
    return magic, acc_mode, groups


def _build_program(n_pos: int, b_is_zero: bool):
    exp_op = _register_exp_op()
    magic, acc_mode, groups = _plan_tiles(n_pos)
    grp_of = {}
    for gi, g in enumerate(groups):
        for slot, t in enumerate(g):
            grp_of[t] = (gi, slot)
    nc = bacc.Bacc()

    # fp8 DoubleRow operands: [ki, ksub, col] with d = ksub*128 + ki.
    xt_t = nc.dram_tensor("xt_t", [128, 2, M], FP8, kind="ExternalInput")
    x_t = nc.dram_tensor("x_t", [128, 2, IC], FP8, kind="ExternalInput")
    cj = nc.dram_tensor("cj", [128, JT], F32, kind="ExternalInput")
    bcol = nc.dram_tensor("bcol", [128, JT], F32, kind="ExternalInput")
    sgn = nc.dram_tensor("sgn", [128, 1], BF16, kind="ExternalInput")
    nxsq = nc.dram_tensor("nxsq", [1, IC], F32, kind="ExternalInput")
    bb = nc.dram_tensor("bb", [1, 1], F32, kind="ExternalInput")
    out = nc.dram_tensor("out", [1, IC], F32, kind="ExternalOutput")
    s_out = nc.dram_tensor("s_out", [1, IC], F32, kind="ExternalOutput")

    NCHUNK = 16           # xt column chunks so matmuls wait on small DMAs
    CW = M // NCHUNK      # 512 j-columns per chunk

    tb = n_pos // 128 if n_pos % 128 else -1   # mixed-sign boundary tile

    def tile_sign(t):
        return +1 if (t + 1) * 128 <= n_pos else -1

    with tile.TileContext(nc) as tc:
        with (
            tc.tile_pool(name="singles", bufs=1) as singles,
            tc.tile_pool(name="epool", bufs=EPOOL) as epool,
            tc.tile_pool(name="gpsum", bufs=3, space="PSUM") as gpsum,
            tc.tile_pool(name="spsum", bufs=1, space="PSUM") as spsum,
        ):
            # Resident inputs, all on the sync HWDGE queue (the scalar queue
            # executes on the ACT engine and steals exp throughput), small
            # first-needed operands ahead of the bulk xt chunks.
            x_sb = singles.tile([128, 2, IC], FP8, tag="x")
            nc.sync.dma_start(out=x_sb, in_=x_t[:, :, :])
            cj_sb = singles.tile([128, JT], F32, tag="cj")
            nc.sync.dma_start(out=cj_sb, in_=cj[:, :])
            bcol_sb = singles.tile([128, JT], F32, tag="bcol")
            nc.sync.dma_start(out=bcol_sb, in_=bcol[:, :])
            sgn_sb = singles.tile([128, 1], BF16, tag="sgn")
            nc.sync.dma_start(out=sgn_sb, in_=sgn[:, :])
            nxsq_sb = singles.tile([1, IC], F32, tag="nxsq")
            nc.sync.dma_start(out=nxsq_sb, in_=nxsq[:, :])
            b_sb = singles.tile([1, 1], F32, tag="b")
            nc.sync.dma_start(out=b_sb, in_=bb[:, :])
            xt_sb = [None] * NCHUNK
            for ck in range(NCHUNK):
                t = singles.tile([128, 2, CW], FP8, tag=f"xt_{ck}")
                nc.sync.dma_start(
                    out=t, in_=xt_t[:, :, ck * CW:(ck + 1) * CW],
                )
                xt_sb[ck] = t

            # Warm the PE while input DMAs are in flight (HAM clock gate).
            # warm_w memset is first so warmups aren't gated on other memsets.
            warm_w = singles.tile([128, 128], BF16, tag="warm_w")
            nc.vector.memset(warm_w, 0.0)
            warm_ps = spsum.tile([1, 128], F32, tag="s")
            for _ in range(N_WARM):
                nc.tensor.matmul(
                    out=warm_ps, lhsT=warm_w[:, 0:1], rhs=warm_w[:, :],
                    start=True, stop=True,
                )

            ones_sb = singles.tile([128, 1], BF16, tag="ones")
            nc.vector.memset(ones_sb, 1.0)
            mones_sb = singles.tile([128, 1], BF16, tag="mones")
            nc.vector.memset(mones_sb, -1.0)
            accs = singles.tile([128, IC], BF16, tag="accs")
            nc.vector.memset(accs, 0.0)
            if groups:
                acc_dp = singles.tile([128, IC], BF16, tag="acc_dp")
                nc.vector.memset(acc_dp, 0.0)
                acc_dn = singles.tile([128, IC], BF16, tag="acc_dn")
                nc.vector.memset(acc_dn, 0.0)

            # Query-side factor, computed early so ACT's table load happens
            # during the DMA window.
            e_row = singles.tile([1, IC], F32, tag="e_row")
            nc.scalar.activation(
                out=e_row, in_=nxsq_sb, func=mybir.ActivationFunctionType.Exp
            )
            # Emulate fp32 FTZ on the factor: the reference's direct
            # exp(-g*d) underflows to 0; keep the factored path bit-identical.
            m_row = singles.tile([1, IC], F32, tag="m_row")
            nc.vector.tensor_scalar(
                out=m_row, in0=e_row, scalar1=FP32_MIN_NORMAL, scalar2=None,
                op0=mybir.AluOpType.is_ge,
            )
            nc.vector.tensor_mul(e_row, e_row, m_row)

            # s_ps accumulates PE-matvec'd tiles across the whole j-loop and
            # receives the folds of acc2/accs at the end. Shares the "s"
            # PSUM slot with warm_ps; the first matvec's start=True clears it.
            s_ps = spsum.tile([1, IC], F32, tag="s")

            e_views = {}          # t -> bf16-view AP of tile t's exp output
            e4_tiles = {}         # group idx -> wide [128, 4, IC] tile
            first_mv = [True]

            def emit_gemm(t):
                ck, col = t // 4, (t % 4) * 128
                g_ps = gpsum.tile([128, IC], F32, tag="g", name=f"g{t}")
                for ic in range(2):
                    sl = slice(ic * 512, (ic + 1) * 512)
                    nc.tensor.matmul(
                        out=g_ps[:, sl],
                        lhsT=xt_sb[ck][:, :, col:col + 128],
                        rhs=x_sb[:, :, sl],
                        start=True, stop=True,
                        perf_mode=mybir.MatmulPerfMode.DoubleRow,
                    )
                return g_ps

            def emit_exp(t, g_ps):
                if t in grp_of:
                    gi, slot = grp_of[t]
                    if gi not in e4_tiles:
                        e4_tiles[gi] = epool.tile(
                            [128, 4, IC], BF16, tag="e4", bufs=2, name=f"e4_{gi}")
                    dest = e4_tiles[gi][:, slot, :]
                else:
                    dest = epool.tile([128, IC], BF16, tag="e", name=f"e{t}")
                if t in magic:
                    nc.vector._custom_dve(
                        exp_op, out=dest.bitcast(U16), in0=g_ps,
                        s0=SCH_A, s1=bcol_sb[:, t:t + 1], imm2=SCH_CLAMP,
                    )
                else:
                    nc.scalar.activation(
                        out=dest, in_=g_ps,
                        func=mybir.ActivationFunctionType.Exp,
                        bias=cj_sb[:, t:t + 1], scale=1.0,
                    )
                e_views[t] = dest

            def emit_acc(t):
                mode = acc_mode[t]
                e_t = e_views.pop(t)
                if mode == "dve":
                    if tile_sign(t) > 0:
                        nc.vector.tensor_add(accs, accs, e_t)
                    else:
                        nc.vector.tensor_sub(accs, accs, e_t)
                elif mode == "grp":
                    gi, slot = grp_of[t]
                    if slot != 3:
                        return     # one CCE DMA per completed group of 4
                    e4 = e4_tiles.pop(gi)
                    dst = acc_dp if tile_sign(t) > 0 else acc_dn
                    nc.gpsimd.dma_start(
                        out=dst.unsqueeze(1).broadcast_to([128, 4, IC]),
                        in_=e4,
                        accum_op=mybir.AluOpType.add,
                    )
                elif mode == "stt":
                    nc.vector.scalar_tensor_tensor(
                        out=accs, in0=e_t, scalar=sgn_sb[:, 0:1], in1=accs,
                        op0=mybir.AluOpType.mult, op1=mybir.AluOpType.add,
                    )
                else:
                    w = ones_sb if tile_sign(t) > 0 else mones_sb
                    for ic in range(2):
                        sl = slice(ic * 512, (ic + 1) * 512)
                        nc.tensor.matmul(
                            out=s_ps[:, sl], lhsT=w, rhs=e_t[:, sl],
                            start=first_mv[0], stop=False,
                            skip_group_check=True,
                        )
                    first_mv[0] = False

            for t in range(JT):
                g_ps = emit_gemm(t)
                emit_exp(t, g_ps)
                if t >= ACC_LAG:
                    emit_acc(t - ACC_LAG)
            for t in range(JT - ACC_LAG, JT):
                emit_acc(t)

            # Fold the DVE/DMA accumulators in: s[0,i] += sum_p acc[p,i];
            # the last matmul stops the PSUM accumulation group.
            folds = [(accs, ones_sb)]
            if groups:
                folds += [(acc_dp, ones_sb), (acc_dn, mones_sb)]
            for fi, (f, w) in enumerate(folds):
                for ic in range(2):
                    sl = slice(ic * 512, (ic + 1) * 512)
                    last = fi == len(folds) - 1 and ic == 1
                    nc.tensor.matmul(
                        out=s_ps[:, sl], lhsT=w, rhs=f[:, sl],
                        start=first_mv[0], stop=last,
                        skip_group_check=True,
                    )
                    first_mv[0] = False
            if DEBUG_S:
                s_sb = singles.tile([1, IC], F32, tag="s_sb")
                nc.vector.tensor_copy(s_sb, s_ps)
                nc.sync.dma_start(out=s_out[:, :], in_=s_sb)  # pre-mask s
            p_row = singles.tile([1, IC], F32, tag="p_row")
            nc.vector.tensor_mul(p_row, s_ps, e_row)
            if not b_is_zero:
                nc.vector.tensor_scalar(
                    out=p_row, in0=p_row, scalar1=b_sb[0:1, 0:1], scalar2=None,
                    op0=mybir.AluOpType.add,
                )
            nc.sync.dma_start(out=out[:, :], in_=p_row)

    nc.finalize()
    return nc


def kernel(X, X_train, alphas, y_train, b):
    X = np.ascontiguousarray(np.asarray(X, dtype=np.float32))
    X_train = np.ascontiguousarray(np.asarray(X_train, dtype=np.float32))
    alphas = np.asarray(alphas, dtype=np.float32).reshape(M)
    y_train = np.asarray(y_train, dtype=np.float32).reshape(M)
    b_arr = np.asarray(b, dtype=np.float32).reshape(1, 1)

    # Sort train points by label (+1 first), then by c within each label so
    # c values on a partition row are close (enables shared-bias tricks).
    c_all = (-GAMMA * (X_train * X_train).sum(1)
             + np.log(np.maximum(alphas, np.float32(1e-38)))).astype(np.float32)
    perm = np.lexsort((c_all, -y_train))
    n_pos = int((y_train > 0).sum())
    Xt_p = X_train[perm]
    c = c_all[perm]

    cj = np.ascontiguousarray(c.reshape(JT, 128).T)          # [128, JT]
    bcol = (SCH_A * cj + np.float32(SCH_B)).astype(np.float32)
    r = n_pos % 128
    sgn_vec = np.where(np.arange(128) < r, 1.0, -1.0).astype(
        ml_dtypes.bfloat16).reshape(128, 1)

    # fp8 DoubleRow layouts: [ki, ksub, col], d = ksub*128 + ki.
    f8 = ml_dtypes.float8_e4m3fn
    xt_dr = np.ascontiguousarray(
        Xt_p.T.reshape(2, 128, M).transpose(1, 0, 2).astype(f8))
    nxsq_full = (-GAMMA * (X * X).sum(1)).astype(np.float32)

    in_maps = []
    for k in range(NCORES):
        sl = slice(k * IC, (k + 1) * IC)
        x_dr = np.ascontiguousarray(
            X[sl].T.reshape(2, 128, IC).transpose(1, 0, 2).astype(f8))
        in_maps.append({
            "xt_t": xt_dr,
            "x_t": x_dr,
            "cj": cj,
            "bcol": bcol,
            "sgn": sgn_vec,
            "nxsq": np.ascontiguousarray(nxsq_full[sl].reshape(1, IC)),
            "bb": b_arr,
        })

    nc = _build_program(n_pos, b_is_zero=float(b_arr.reshape(-1)[0]) == 0.0)
    res = run_bass_kernel_spmd(nc, in_maps, list(range(NCORES)), trace=TRACE)
    global LAST_RESULTS
    LAST_RESULTS = res

    preds = np.concatenate([res.results[k]["out"][0] for k in range(NCORES)])
    return preds.reshape(N, 1).astype(np.float32)
